# revision 1
# baseline (speedup 1.0000x reference)
# Trainium2 Bass kernel for DeepSeek-style sparse attention.
# Self-contained: hardcodes shapes from the problem spec.
#   x [1, 2048, 768]; Wq/Wk/Wv/Wo [768, 768]; biases [768]; Ws [12, 768]; bs [12]
# Strategy: row-shard the 2048 query positions across 8 cores (256 rows each).
# Each core redundantly computes full K/V projections from a (per-core
# column-rotated) copy of x^T, so no collectives are needed. Three sparse
# attention branches (local band / learned top-k / global) are evaluated from
# one dense exp(S^T) per head:
#   - top-k: column mask folded into V (E @ (m*v)), mask from a tiny phase-A
#     token-score kernel + host argpartition between the two NEFF launches.
#   - local band: per-core rotation puts each core's 640-wide band in t-chunks
#     0..5; a host-built 0/1 mask is applied to E^T before a 6-chunk matmul.
#   - global (first 16 tokens): separate tiny k/v path from the unrotated
#     first 16 columns of x (uniform across cores despite the rotation).
# Matmuls run as float32r (TF32-like, 4x faster than fp32 for N>=256).
import sys
import numpy as np
import ml_dtypes

sys.path.insert(0, "/opt/trn_rl_repo")

import concourse.bass as bass
from concourse import bacc
import concourse.mybir as mybir
from concourse.tile import TileContext
from concourse.bass_utils import run_bass_kernel_spmd

S = 2048
D = 768
H = 12
DH = 64
NCORES = 8
RPC = S // NCORES          # 256 query rows per core
NCH = S // 128             # 16 t-chunks
ECH = D // 128             # 6 embedding chunks
TOPK = 256
NG = 16
LWH = 256                  # local window half-width
SCALE = 1.0 / np.sqrt(DH)
F32 = mybir.dt.float32
F32R = mybir.dt.float32r
BF16 = mybir.dt.bfloat16


def _patch_tile_drain():
    """This walrus build rejects sem-waits on Drain instructions ("Too many
    sync wait commands"). Emit the tail waits as individual SemWait ops on
    the sync engine instead, then a bare drain."""
    if getattr(TileContext, "_drain_patched", False):
        return

    def _drain_and_barrier(self, tick_clock, wait_clock):
        nc = self.nc
        clock = tick_clock.global_clock
        for proc, handle in sorted(self.sems.allocated().items()):
            tick = clock[proc]
            if tick <= 0:
                continue
            mult = 16 if "DMA" in handle.name else 1
            nc.sync.wait_ge(handle, tick * mult)
        nc.sync.drain()
        nc.all_engine_barrier()
        popped = nc._tile_sem_poison_stack.pop()
        assert popped is self._sem_poison
        nc.clear_and_free_semaphores(list(self.sems.allocated().values()))
        nc.all_engine_barrier()

    TileContext._drain_and_barrier = _drain_and_barrier
    TileContext._drain_patched = True


def _build_phase_a():
    """ts[h, t] = (Ws @ x^T + bs)[h, t] on one core, plain fp32."""
    nc = bacc.Bacc()
    xT = nc.declare_dram_parameter("xT", [D, S], F32, isOutput=False)
    WsT = nc.declare_dram_parameter("WsT", [D, H], F32, isOutput=False)
    bs_row = nc.declare_dram_parameter("bs_row", [1, H], F32, isOutput=False)
    ts = nc.declare_dram_parameter("ts", [H, S], F32, isOutput=True)
    xT_r = xT.rearrange("(c p) t -> c p t", p=128)
    WsT_r = WsT.rearrange("(c p) h -> c p h", p=128)

    with TileContext(nc) as tc:
        with (
            tc.tile_pool(name="sb", bufs=1) as sb,
            tc.tile_pool(name="ps", bufs=2, space="PSUM") as ps,
        ):
            xT_sb = sb.tile([128, ECH, S], F32)
            WsT_sb = sb.tile([128, ECH, H], F32)
            ones = sb.tile([1, 512], F32)
            bs_sb = sb.tile([1, H], F32)
            nc.vector.memset(ones, 1.0)
            nc.sync.dma_start(out=bs_sb, in_=bs_row[:, :])
            for ec in range(ECH):
                nc.sync.dma_start(out=xT_sb[:, ec, :], in_=xT_r[ec])
                nc.sync.dma_start(out=WsT_sb[:, ec, :], in_=WsT_r[ec])
            ts_sb = sb.tile([H, S], F32)
            for t4 in range(4):
                acc = ps.tile([H, 512], F32)
                for ec in range(ECH):
                    nc.tensor.matmul(
                        acc,
                        WsT_sb[:, ec, :],
                        xT_sb[:, ec, 512 * t4 : 512 * (t4 + 1)],
                        start=(ec == 0),
                        stop=False,
                    )
                nc.tensor.matmul(acc, bs_sb, ones, start=False, stop=True)
                nc.vector.tensor_copy(ts_sb[:, 512 * t4 : 512 * (t4 + 1)], acc)
            nc.sync.dma_start(out=ts[:, :], in_=ts_sb)
    nc.finalize()
    return nc


def _build_phase_b():
    """Per-core attention kernel. Query rows [c*256, (c+1)*256); t-columns of
    all per-t tensors are rotated left by 128*i0u(c) so the local band always
    occupies rotated t-chunks 0..5."""
    nc = bacc.Bacc()
    xTr = nc.declare_dram_parameter("xTr", [D, S], F32R, isOutput=False)
    xTq = nc.declare_dram_parameter("xTq", [D, RPC], F32R, isOutput=False)
    xTg = nc.declare_dram_parameter("xTg", [D, NG], F32R, isOutput=False)
    WqT = nc.declare_dram_parameter("WqT", [D, D], F32R, isOutput=False)
    WkT = nc.declare_dram_parameter("WkT", [D, D], F32R, isOutput=False)
    WvT = nc.declare_dram_parameter("WvT", [D, D], F32R, isOutput=False)
    WoT = nc.declare_dram_parameter("WoT", [H, DH, D], F32R, isOutput=False)
    bq_r = nc.declare_dram_parameter("bq_r", [1, D], F32R, isOutput=False)
    bk_r = nc.declare_dram_parameter("bk_r", [1, D], F32R, isOutput=False)
    bv_r = nc.declare_dram_parameter("bv_r", [1, D], F32R, isOutput=False)
    bo_r = nc.declare_dram_parameter("bo_r", [1, D], F32R, isOutput=False)
    M6 = nc.declare_dram_parameter("M6", [128, 6, RPC], BF16, isOutput=False)
    tkm = nc.declare_dram_parameter("tkm", [128, NCH, H], F32, isOutput=False)
    yT = nc.declare_dram_parameter("yT", [D, RPC], F32, isOutput=True)

    xTr_p = xTr.rearrange("(c p) t -> p c t", p=128)
    xTq_p = xTq.rearrange("(c p) t -> p c t", p=128)
    xTg_p = xTg.rearrange("(c p) t -> p c t", p=128)
    WqT_p = WqT.rearrange("(c p) d -> p c d", p=128)
    WkT_p = WkT.rearrange("(c p) d -> p c d", p=128)
    WvT_p = WvT.rearrange("(c p) d -> p c d", p=128)
    WoT_p = WoT.rearrange("h p d -> p h d")
    yT_p = yT.rearrange("(c p) t -> p c t", p=128)
    Exp = mybir.ActivationFunctionType.Exp

    with TileContext(nc) as tc, nc.allow_low_precision(reason="tf32/bf16 validated vs reference"):
        with tc.tile_pool(name="perm", bufs=1) as perm:
            kT_sb = perm.tile([128, ECH, S], F32R)
            kTg_sb = perm.tile([128, ECH, NG], F32R)
            v_sb = perm.tile([128, NCH, H, DH + 1], BF16)
            vg_sb = perm.tile([NG, H, DH + 1], BF16)
            qT_sb = perm.tile([128, ECH, RPC], F32R)
            tkm_sb = perm.tile([128, NCH, H], F32)
            M6_sb = perm.tile([128, 6, RPC], BF16)
            attnT_sb = perm.tile([DH, H, RPC], F32R)
            yT_sb = perm.tile([128, ECH, RPC], F32)
            ones = perm.tile([1, 512], F32R)
            ones65 = perm.tile([DH + 1, DH, ], F32R)
            bq_sb = perm.tile([1, D], F32R)
            bk_sb = perm.tile([1, D], F32R)
            bv_sb = perm.tile([1, D], F32R)
            bo_sb = perm.tile([1, D], F32R)
            onesf = perm.tile([DH + 1, 512], F32)
            nc.vector.memset(onesf, 1.0)
            nc.vector.tensor_copy(ones, onesf[0:1, :])
            nc.vector.tensor_copy(ones65, onesf[:, 0:DH])
            nc.vector.memset(v_sb, 1.0)
            nc.vector.memset(vg_sb, 1.0)
            nc.sync.dma_start(out=tkm_sb, in_=tkm[:, :, :])
            nc.sync.dma_start(out=M6_sb, in_=M6[:, :, :])
            for t, d in ((bq_sb, bq_r), (bk_sb, bk_r), (bv_sb, bv_r), (bo_sb, bo_r)):
                nc.sync.dma_start(out=t, in_=d[:, :])

            with (
                tc.tile_pool(name="xin", bufs=1) as xin,
                tc.tile_pool(name="pj_ps", bufs=4, space="PSUM") as pj_ps,
            ):
                xTr_sb = xin.tile([128, ECH, S], F32R)
                xTq_sb = xin.tile([128, ECH, RPC], F32R)
                xTg_sb = xin.tile([128, ECH, NG], F32R)
                nc.sync.dma_start(out=xTr_sb, in_=xTr_p)
                nc.sync.dma_start(out=xTq_sb, in_=xTq_p)
                nc.sync.dma_start(out=xTg_sb, in_=xTg_p)

                # ---- V projection (v natural [t, h, dh], +ones col) ----
                with tc.tile_pool(name="vw", bufs=1) as vw:
                    WvT_sb = vw.tile([128, ECH, D], F32R)
                    nc.sync.dma_start(out=WvT_sb, in_=WvT_p)
                    for tcn in range(NCH):
                        for half, (v0, v1) in enumerate(((0, 512), (512, 768))):
                            vn = v1 - v0
                            vp = pj_ps.tile([128, 512], F32, tag="pj")
                            for ec in range(ECH):
                                nc.tensor.matmul(
                                    vp[:, :vn],
                                    xTr_sb[:, ec, 128 * tcn : 128 * (tcn + 1)],
                                    WvT_sb[:, ec, v0:v1],
                                    start=(ec == 0), stop=False,
                                )
                            nc.tensor.matmul(
                                vp[:, :vn], ones[:, :128],
                                bv_sb[:, v0:v1], start=False, stop=True,
                            )
                            h0 = 0 if half == 0 else 8
                            nc.any.tensor_copy(
                                v_sb[:, tcn, h0 : h0 + vn // DH, 0:DH],
                                vp[:, :vn].rearrange("p (h d) -> p h d", d=DH),
                            )
                    # vg (first NG unrotated tokens)
                    for half, (v0, v1) in enumerate(((0, 512), (512, 768))):
                        vn = v1 - v0
                        vp = pj_ps.tile([128, 512], F32, tag="pj")
                        for ec in range(ECH):
                            nc.tensor.matmul(
                                vp[:NG, :vn], xTg_sb[:, ec, :],
                                WvT_sb[:, ec, v0:v1],
                                start=(ec == 0), stop=False,
                            )
                        nc.tensor.matmul(
                            vp[:NG, :vn], ones[:, :NG],
                            bv_sb[:, v0:v1], start=False, stop=True,
                        )
                        h0 = 0 if half == 0 else 8
                        nc.any.tensor_copy(
                            vg_sb[:, h0 : h0 + vn // DH, 0:DH],
                            vp[:NG, :vn].rearrange("p (h d) -> p h d", d=DH),
                        )

                # ---- K^T, K^T-global, Q^T projections ----
                with tc.tile_pool(name="kw", bufs=2) as kw:
                    for dc in range(ECH):
                        wk = kw.tile([128, ECH, 128], F32R, tag="wk")
                        wq = kw.tile([128, ECH, 128], F32R, tag="wq")
                        nc.sync.dma_start(out=wk, in_=WkT_p[:, :, 128 * dc : 128 * (dc + 1)])
                        nc.sync.dma_start(out=wq, in_=WqT_p[:, :, 128 * dc : 128 * (dc + 1)])
                        for t4 in range(4):
                            kp = pj_ps.tile([128, 512], F32, tag="pj")
                            for ec in range(ECH):
                                nc.tensor.matmul(
                                    kp, wk[:, ec, :],
                                    xTr_sb[:, ec, 512 * t4 : 512 * (t4 + 1)],
                                    start=(ec == 0), stop=False,
                                )
                            nc.tensor.matmul(
                                kp, bk_sb[:, 128 * dc : 128 * (dc + 1)],
                                ones[:, :512], start=False, stop=True,
                            )
                            nc.any.tensor_copy(kT_sb[:, dc, 512 * t4 : 512 * (t4 + 1)], kp)
                        kgp = pj_ps.tile([128, 512], F32, tag="pj")
                        for ec in range(ECH):
                            nc.tensor.matmul(
                                kgp[:, :NG], wk[:, ec, :], xTg_sb[:, ec, :],
                                start=(ec == 0), stop=False,
                            )
                        nc.tensor.matmul(
                            kgp[:, :NG], bk_sb[:, 128 * dc : 128 * (dc + 1)],
                            ones[:, :NG], start=False, stop=True,
                        )
                        nc.any.tensor_copy(kTg_sb[:, dc, :], kgp[:, :NG])
                        qp = pj_ps.tile([128, 512], F32, tag="pj")
                        for ec in range(ECH):
                            nc.tensor.matmul(
                                qp[:, :RPC], wq[:, ec, :],
                                xTq_sb[:, ec, :],
                                start=(ec == 0), stop=False,
                            )
                        nc.tensor.matmul(
                            qp[:, :RPC], bq_sb[:, 128 * dc : 128 * (dc + 1)],
                            ones[:, :RPC], start=False, stop=True,
                        )
                        nc.any.tensor_copy(qT_sb[:, dc, :], qp[:, :RPC])

            # ---- per-head attention ----
            with (
                tc.tile_pool(name="attn", bufs=2) as attn,
                tc.tile_pool(name="one_sb", bufs=1) as one_sb,
                tc.tile_pool(name="st_ps", bufs=2, space="PSUM") as st_ps,
                tc.tile_pool(name="av_ps", bufs=1, space="PSUM") as av_ps,
                tc.tile_pool(name="ms_ps", bufs=1, space="PSUM") as ms_ps,
            ):
                for h in range(H):
                    dc, hp = h // 2, (h % 2) * 64
                    kTh = kT_sb[hp : hp + 64, dc, :]
                    qTh = qT_sb[hp : hp + 64, dc, :]
                    ET = attn.tile([128, NCH, RPC], BF16, tag="ET")
                    for rnd in range(4):
                        stp = st_ps.tile([128, 4, RPC], F32, tag="st")
                        for j in range(4):
                            i = rnd * 4 + j
                            nc.tensor.matmul(
                                stp[:, j, :],
                                kTh[:, 128 * i : 128 * (i + 1)],
                                qTh, start=True, stop=True,
                            )
                        nc.scalar.activation(
                            ET[:, 4 * rnd : 4 * (rnd + 1), :], stp, Exp, scale=SCALE
                        )
                    stg = ms_ps.tile([64, 3, RPC], F32, tag="ms")
                    nc.tensor.matmul(
                        stg[:NG, 0, :], kTg_sb[hp : hp + 64, dc, :],
                        qTh, start=True, stop=True,
                    )
                    ETg = attn.tile([NG, RPC], BF16, tag="ETg")
                    nc.scalar.activation(ETg, stg[:NG, 0, :], Exp, scale=SCALE)
                    vm = attn.tile([128, NCH, DH + 1], BF16, tag="vm")
                    for i in range(NCH):
                        nc.vector.tensor_scalar_mul(
                            vm[:, i, :], v_sb[:, i, h, :], tkm_sb[:, i, h : h + 1]
                        )
                    EB = attn.tile([128, 6, RPC], BF16, tag="EB")
                    nc.vector.tensor_mul(EB, ET[:, 0:6, :], M6_sb)
                    av = av_ps.tile([128, 3, RPC], F32, tag="av")
                    for i in range(NCH):
                        nc.tensor.matmul(
                            av[0:65, 0, :], vm[:, i, :], ET[:, i, :],
                            start=(i == 0), stop=(i == NCH - 1),
                        )
                    for k in range(6):
                        nc.tensor.matmul(
                            av[0:65, 1, :], v_sb[:, k, h, :], EB[:, k, :],
                            start=(k == 0), stop=(k == 5),
                        )
                    nc.tensor.matmul(
                        av[0:65, 2, :], vg_sb[:, h, :], ETg, start=True, stop=True,
                    )
                    sums = attn.tile([DH + 1, 3, RPC], F32, tag="sums")
                    nc.vector.tensor_scalar_mul(
                        sums[DH : DH + 1, :, :], av[DH : DH + 1, :, :], 3.0
                    )
                    rin = attn.tile([DH + 1, 3, RPC], F32R, tag="rin")
                    nc.vector.reciprocal(
                        rin[DH : DH + 1, :, :], sums[DH : DH + 1, :, :]
                    )
                    rbc = ms_ps.tile([64, 3, RPC], F32, tag="ms")
                    for b in range(3):
                        nc.tensor.matmul(
                            rbc[:, b, :], ones65[DH : DH + 1, :],
                            rin[DH : DH + 1, b, :], start=True, stop=True,
                        )
                    rbs = attn.tile([64, 3, RPC], F32, tag="rbs")
                    nc.vector.tensor_copy(rbs, rbc)
                    acc = attnT_sb[:, h, :]
                    tmp = attn.tile([64, RPC], F32, tag="tmp")
                    nc.vector.tensor_mul(acc, av[0:64, 0, :], rbs[:, 0, :])
                    nc.vector.tensor_mul(tmp, av[0:64, 1, :], rbs[:, 1, :])
                    nc.vector.tensor_add(acc, acc, tmp)
                    nc.vector.tensor_mul(tmp, av[0:64, 2, :], rbs[:, 2, :])
                    nc.vector.tensor_add(acc, acc, tmp)

            # ---- output projection yT = WoT.T @ attnT + bo ----
            with (
                tc.tile_pool(name="wo", bufs=2) as wo_pool,
                tc.tile_pool(name="yt_ps", bufs=2, space="PSUM") as yt_ps,
            ):
                for dc in range(ECH):
                    wo = wo_pool.tile([DH, H, 128], F32R, tag="wo")
                    nc.sync.dma_start(out=wo, in_=WoT_p[:, :, 128 * dc : 128 * (dc + 1)])
                    yp = yt_ps.tile([128, RPC], F32, tag="yt")
                    for h in range(H):
                        nc.tensor.matmul(
                            yp, wo[:, h, :],
                            attnT_sb[:, h, :],
                            start=(h == 0), stop=False,
                        )
                    nc.tensor.matmul(
                        yp, bo_sb[:, 128 * dc : 128 * (dc + 1)],
                        ones[:, :RPC], start=False, stop=True,
                    )
                    nc.any.tensor_copy(yT_sb[:, dc, :], yp)
                    nc.sync.dma_start(out=yT_p[:, dc, :], in_=yT_sb[:, dc, :])
    nc.finalize()
    return nc


_PROGS = {}
TRACE = False
LAST_EXEC_NS = {}


def _get_progs():
    if "a" not in _PROGS:
        _PROGS["a"] = _build_phase_a()
        _PROGS["b"] = _build_phase_b()
    return _PROGS["a"], _PROGS["b"]


def _band_mask(c):
    i0u = min(max(2 * c - 2, 0), 10)
    r0 = c * RPC
    p = np.arange(128)[:, None, None]
    k = np.arange(6)[None, :, None]
    sl = np.arange(RPC)[None, None, :]
    t = 128 * (i0u + k) + p
    sg = r0 + sl
    return (np.abs(t - sg) <= LWH).astype(np.float32), i0u


def kernel(**inputs):
    x = np.ascontiguousarray(inputs["x"][0], np.float32)        # [S, D]
    xT = np.ascontiguousarray(x.T)                              # [D, S]
    nc_a, nc_b = _get_progs()

    # phase A: token scores on core 0
    in_a = {
        "xT": xT,
        "WsT": np.ascontiguousarray(inputs["Ws"].T, np.float32),
        "bs_row": np.ascontiguousarray(inputs["bs"][None, :], np.float32),
    }
    ra = run_bass_kernel_spmd(nc_a, [in_a], [0], trace=TRACE)
    ts = ra.results[0]["ts"]  # [H, S]
    LAST_EXEC_NS["phase_a"] = ra.exec_time_ns

    # host: top-k column mask per head
    tkm = np.zeros((H, S), np.float32)
    for h in range(H):
        tkm[h, np.argpartition(-ts[h], TOPK)[:TOPK]] = 1.0

    shared = {
        "WqT": np.ascontiguousarray(inputs["Wq"].T, np.float32),
        "WkT": np.ascontiguousarray(inputs["Wk"].T, np.float32),
        "WvT": np.ascontiguousarray(inputs["Wv"].T, np.float32),
        "WoT": np.ascontiguousarray(inputs["Wo"].T, np.float32).reshape(H, DH, D),
        "bq_r": np.ascontiguousarray(inputs["bq"][None, :], np.float32),
        "bk_r": np.ascontiguousarray(inputs["bk"][None, :], np.float32),
        "bv_r": np.ascontiguousarray(inputs["bv"][None, :], np.float32),
        "bo_r": np.ascontiguousarray(inputs["bo"][None, :], np.float32),
        "xTg": np.ascontiguousarray(xT[:, :NG]),
    }
    in_maps = []
    for c in range(NCORES):
        M6, i0u = _band_mask(c)
        rot = np.roll(xT, -128 * i0u, axis=1)
        tkm_rot = np.roll(tkm, -128 * i0u, axis=1)
        tkm_r = np.ascontiguousarray(
            tkm_rot.reshape(H, NCH, 128).transpose(2, 1, 0), np.float32
        )
        in_maps.append(dict(
            shared,
            xTr=np.ascontiguousarray(rot),
            xTq=np.ascontiguousarray(xT[:, c * RPC : (c + 1) * RPC]),
            M6=np.ascontiguousarray(M6.astype(ml_dtypes.bfloat16)),
            tkm=tkm_r,
        ))
    res = run_bass_kernel_spmd(nc_b, in_maps, list(range(NCORES)), trace=TRACE)
    LAST_EXEC_NS["phase_b"] = res.exec_time_ns
    out = np.empty((S, D), np.float32)
    for c in range(NCORES):
        out[c * RPC : (c + 1) * RPC] = res.results[c]["yT"].T
    return out.reshape(1, S, D)



# revision 23
# speedup vs baseline: 1.6369x; 1.6369x over previous
# Trainium2 Bass kernel for DeepSeek-style sparse attention.
# Self-contained: hardcodes shapes from the problem spec.
#   x [1, 2048, 768]; Wq/Wk/Wv/Wo [768, 768]; biases [768]; Ws [12, 768]; bs [12]
#
# Sharding: 8 cores = 4 query-blocks (512 queries) x 2 head-groups (6 heads).
# Each core computes, for its 512 queries and 6 heads:
#   - band K/Q/V projections over an 8-chunk (1024-token) window that covers
#     the +-256 local band of its query block (host slices x accordingly),
#   - a compact top-k branch from host-gathered x columns (256 per head,
#     indices from a tiny fp32 phase-A token-score kernel + host argpartition),
#   - the 16-token global branch,
#   - a partial out-projection over its 6 heads' dims. The host sums the two
#     head-group partials per query block and adds bo (no device collective).
# Everything post-PSUM runs in bf16; phase A stays fp32 so the top-k
# selection matches the fp32 reference bit-for-bit on near-ties.
import sys
import numpy as np
import ml_dtypes

sys.path.insert(0, "/opt/trn_rl_repo")

import concourse.bass as bass
from concourse import bacc
import concourse.mybir as mybir
from concourse.tile import TileContext
from concourse.bass_utils import run_bass_kernel_spmd
from concourse.masks import make_identity

S = 2048
D = 768
H = 12
DH = 64
NCORES = 8
NQB = 4                    # query blocks
QB = S // NQB              # 512 queries per block
HG = 2                     # head groups
HPG = H // HG              # 6 heads per group
ECH = D // 128             # 6 embedding chunks
BCH = 8                    # band chunks per core (8 x 128 = 1024 tokens)
BT = BCH * 128             # 1024 band tokens
TOPK = 256
NG = 16
LWH = 256                  # local window half-width
SCALE = 1.0 / np.sqrt(DH)
F32 = mybir.dt.float32
BF16 = mybir.dt.bfloat16
Exp = mybir.ActivationFunctionType.Exp


def _patch_tile_drain():
    """This walrus build rejects sem-waits on Drain instructions ("Too many
    sync wait commands"). Emit the tail waits as individual SemWait ops on
    the sync engine instead, then a bare drain."""
    if getattr(TileContext, "_drain_patched", False):
        return

    def _drain_and_barrier(self, tick_clock, wait_clock):
        nc = self.nc
        clock = tick_clock.global_clock
        for proc, handle in sorted(self.sems.allocated().items()):
            tick = clock[proc]
            if tick <= 0:
                continue
            mult = 16 if "DMA" in handle.name else 1
            nc.sync.wait_ge(handle, tick * mult)
        nc.sync.drain()
        nc.all_engine_barrier()
        popped = nc._tile_sem_poison_stack.pop()
        assert popped is self._sem_poison
        nc.clear_and_free_semaphores(list(self.sems.allocated().values()))
        nc.all_engine_barrier()

    TileContext._drain_and_barrier = _drain_and_barrier
    TileContext._drain_patched = True


def _build_phase_a():
    """ts[h, 256] = (Ws @ x^T + bs) for this core's 256-token slice, fp32."""
    _patch_tile_drain()
    nc = bacc.Bacc()
    TPC = S // NCORES  # 256 tokens per core
    xTa = nc.declare_dram_parameter("xTa", [128, ECH, TPC], F32, isOutput=False)
    WsT = nc.declare_dram_parameter("WsT", [128, ECH, H], F32, isOutput=False)
    bs_row = nc.declare_dram_parameter("bs_row", [1, H], F32, isOutput=False)
    ts = nc.declare_dram_parameter("ts", [H, TPC], F32, isOutput=True)

    with TileContext(nc) as tc:
        with (
            tc.tile_pool(name="sb", bufs=1) as sb,
            tc.tile_pool(name="ps", bufs=1, space="PSUM") as ps,
        ):
            xTa_sb = sb.tile([128, ECH, TPC], F32)
            WsT_sb = sb.tile([128, ECH, H], F32)
            bs_sb = sb.tile([1, H], F32)
            ones = sb.tile([1, TPC], F32)
            nc.vector.memset(ones, 1.0)
            nc.sync.dma_start(out=bs_sb, in_=bs_row[:, :])
            nc.sync.dma_start(out=WsT_sb, in_=WsT[:, :, :])
            nc.sync.dma_start(out=xTa_sb, in_=xTa[:, :, :])
            acc = ps.tile([H, TPC], F32)
            for ec in range(ECH):
                nc.tensor.matmul(
                    acc, WsT_sb[:, ec, :], xTa_sb[:, ec, :],
                    start=(ec == 0), stop=False,
                )
            nc.tensor.matmul(acc, bs_sb, ones, start=False, stop=True)
            ts_sb = sb.tile([H, TPC], F32)
            nc.vector.tensor_copy(ts_sb, acc)
            nc.sync.dma_start(out=ts[:, :], in_=ts_sb)
    nc.finalize()
    return nc


def _build_phase_b(debug=False):
    """Per-core sparse attention for 512 queries x 6 heads (see header)."""
    _patch_tile_drain()
    nc = bacc.Bacc()
    if debug:
        davs = nc.declare_dram_parameter("davs", [DH + 1, 3, QB], F32, isOutput=True)
        drin = nc.declare_dram_parameter("drin", [1, 3, QB], F32, isOutput=True)
        drin0 = nc.declare_dram_parameter("drin0", [1, 3, QB], F32, isOutput=True)
        drbs = nc.declare_dram_parameter("drbs", [DH, 3, QB], F32, isOutput=True)
        dattnT = nc.declare_dram_parameter("dattnT", [DH, QB], F32, isOutput=True)
    # host-prepared, partition-major, bf16
    xTb = nc.declare_dram_parameter("xTb", [128, ECH, BT], BF16, isOutput=False)
    xTq = nc.declare_dram_parameter("xTq", [128, ECH, QB], BF16, isOutput=False)
    xTk = nc.declare_dram_parameter("xTk", [128, ECH, HPG * TOPK], BF16, isOutput=False)
    xTg = nc.declare_dram_parameter("xTg", [128, ECH, NG], BF16, isOutput=False)
    # per-type weight slices for this head group (384 = 6 heads x 64 dims)
    WqTg = nc.declare_dram_parameter("WqTg", [128, ECH, 384], BF16, isOutput=False)
    WkTg = nc.declare_dram_parameter("WkTg", [128, ECH, 384], BF16, isOutput=False)
    WvTg = nc.declare_dram_parameter("WvTg", [128, ECH, 384], BF16, isOutput=False)
    bqg = nc.declare_dram_parameter("bqg", [1, 384], BF16, isOutput=False)
    bkg = nc.declare_dram_parameter("bkg", [1, 384], BF16, isOutput=False)
    bvg = nc.declare_dram_parameter("bvg", [1, 384], BF16, isOutput=False)
    # stacked [Wk_h | Wv_h] (even local head) / [Wv_h | Wk_h] (odd) for the
    # compact top-k / global projections
    Wkv2 = nc.declare_dram_parameter("Wkv2", [128, ECH, HPG, 128], BF16, isOutput=False)
    bkv2 = nc.declare_dram_parameter("bkv2", [1, HPG, 128], BF16, isOutput=False)
    # Wo^T/3 slices: wo[dh, j, dc, :] over this group's heads
    Wo3 = nc.declare_dram_parameter("Wo3", [DH, HPG, ECH, 128], BF16, isOutput=False)
    M8 = nc.declare_dram_parameter("M8", [128, BCH, QB], BF16, isOutput=False)
    yT = nc.declare_dram_parameter("yT", [128, ECH, QB], F32, isOutput=True)

    with TileContext(nc) as tc, nc.allow_low_precision(reason="bf16 validated vs reference"):
        with tc.tile_pool(name="perm", bufs=1) as perm:
            kT_sb = perm.tile([128, HPG // 2, BT], BF16)       # 2 heads / 128 parts
            qT_sb = perm.tile([128, HPG // 2, QB], BF16)
            ktk_sb = perm.tile([128, HPG // 2, TOPK], BF16)
            vtkT_sb = perm.tile([128, HPG // 2, TOPK], BF16)   # staging (pre-transpose)
            vtk_sb = perm.tile([128, TOPK // 128, HPG, DH + 1], BF16)
            kg_sb = perm.tile([128, HPG // 2, NG], BF16)
            vgT_sb = perm.tile([128, HPG // 2, NG], BF16)      # staging
            vg_sb = perm.tile([NG, HPG, DH + 1], BF16)
            v_sb = perm.tile([128, BCH, HPG, DH + 1], BF16)
            M8_sb = perm.tile([128, BCH, QB], BF16)
            attnT = perm.tile([DH, HPG, QB], BF16)
            yT_sb = perm.tile([128, ECH, QB], F32)
            wo_sb = perm.tile([DH, HPG, ECH, 128], BF16)
            bqg_sb = perm.tile([1, 384], BF16)
            bkg_sb = perm.tile([1, 384], BF16)
            bvg_sb = perm.tile([1, 384], BF16)
            bkv2_sb = perm.tile([1, HPG, 128], BF16)
            ones = perm.tile([1, QB], BF16)
            id128 = perm.tile([128, 128], BF16)
            nc.vector.memset(ones, 1.0)
            make_identity(nc, id128)
            nc.vector.memset(v_sb[:, :, :, DH : DH + 1], 1.0)
            nc.vector.memset(vtk_sb[:, :, :, DH : DH + 1], 1.0)
            nc.vector.memset(vg_sb[:, :, DH : DH + 1], 1.0)
            nc.sync.dma_start(out=M8_sb, in_=M8[:, :, :])
            nc.sync.dma_start(out=bqg_sb, in_=bqg[:, :])
            nc.sync.dma_start(out=bkg_sb, in_=bkg[:, :])
            nc.sync.dma_start(out=bvg_sb, in_=bvg[:, :])
            nc.sync.dma_start(out=bkv2_sb, in_=bkv2[:, :, :])
            nc.sync.dma_start(out=wo_sb, in_=Wo3[:, :, :, :])

            with (
                tc.tile_pool(name="xin", bufs=1) as xin,
                tc.tile_pool(name="pj_ps", bufs=4, space="PSUM") as pj_ps,
                tc.tile_pool(name="pg_ps", bufs=2, space="PSUM") as pg_ps,
                tc.tile_pool(name="pt_ps", bufs=2, space="PSUM") as pt_ps,
            ):
                xTb_sb = xin.tile([128, ECH, BT], BF16)
                xTq_sb = xin.tile([128, ECH, QB], BF16)
                xTk_sb = xin.tile([128, ECH, HPG * TOPK], BF16)
                xTg_sb = xin.tile([128, ECH, NG], BF16)
                WqTg_sb = xin.tile([128, ECH, 384], BF16)
                WkTg_sb = xin.tile([128, ECH, 384], BF16)
                WvTg_sb = xin.tile([128, ECH, 384], BF16)
                Wkv2_sb = xin.tile([128, ECH, HPG, 128], BF16)
                for ec in range(ECH):
                    nc.sync.dma_start(out=WkTg_sb[:, ec, :], in_=WkTg[:, ec])
                    nc.sync.dma_start(out=WqTg_sb[:, ec, :], in_=WqTg[:, ec])
                    nc.sync.dma_start(out=WvTg_sb[:, ec, :], in_=WvTg[:, ec])
                    nc.sync.dma_start(out=xTb_sb[:, ec, :], in_=xTb[:, ec])
                for ec in range(ECH):
                    nc.sync.dma_start(out=xTq_sb[:, ec, :], in_=xTq[:, ec])
                    nc.sync.dma_start(out=xTk_sb[:, ec, :], in_=xTk[:, ec])
                    nc.sync.dma_start(out=Wkv2_sb[:, ec], in_=Wkv2[:, ec])
                nc.sync.dma_start(out=xTg_sb, in_=xTg[:, :, :])

                # ---- band K^T [dims, 1024] and Q^T [dims, 512] ----
                for dc in range(HPG // 2):
                    ds = slice(128 * dc, 128 * (dc + 1))
                    for half in range(2):
                        t0 = 512 * half
                        kp = pj_ps.tile([128, 512], F32, tag="pj")
                        for ec in range(ECH):
                            nc.tensor.matmul(
                                kp, WkTg_sb[:, ec, ds],
                                xTb_sb[:, ec, t0 : t0 + 512],
                                start=(ec == 0), stop=False,
                            )
                        nc.tensor.matmul(
                            kp, bkg_sb[:, ds], ones, start=False, stop=True
                        )
                        nc.any.tensor_copy(kT_sb[:, dc, t0 : t0 + 512], kp)
                    qp = pj_ps.tile([128, 512], F32, tag="pj")
                    for ec in range(ECH):
                        nc.tensor.matmul(
                            qp, WqTg_sb[:, ec, ds], xTq_sb[:, ec, :],
                            start=(ec == 0), stop=False,
                        )
                    nc.tensor.matmul(
                        qp, bqg_sb[:, ds], ones, start=False, stop=True
                    )
                    nc.any.tensor_copy(qT_sb[:, dc, :], qp)

                # ---- band V [1024 tokens, 6 heads x 64] ----
                for tcn in range(BCH):
                    t0 = 128 * tcn
                    vp = pj_ps.tile([128, HPG * DH], F32, tag="pj")
                    for ec in range(ECH):
                        nc.tensor.matmul(
                            vp, xTb_sb[:, ec, t0 : t0 + 128],
                            WvTg_sb[:, ec, :],
                            start=(ec == 0), stop=False,
                        )
                    nc.tensor.matmul(
                        vp, ones[:, :128], bvg_sb,
                        start=False, stop=True,
                    )
                    nc.any.tensor_copy(
                        v_sb[:, tcn, :, 0:DH],
                        vp.rearrange("p (h d) -> p h d", d=DH),
                    )

                # ---- compact top-k + global K/V per head (stacked K|V) ----
                for j in range(HPG):
                    hp = (j % 2) * 64
                    dc = j // 2
                    sp = pj_ps.tile([128, TOPK], F32, tag="pj")
                    sg = pg_ps.tile([128, NG], F32, tag="pg")
                    for ec in range(ECH):
                        nc.tensor.matmul(
                            sp, Wkv2_sb[:, ec, j, :],
                            xTk_sb[:, ec, TOPK * j : TOPK * (j + 1)],
                            start=(ec == 0), stop=False,
                        )
                        nc.tensor.matmul(
                            sg, Wkv2_sb[:, ec, j, :], xTg_sb[:, ec, :],
                            start=(ec == 0), stop=False,
                        )
                    nc.tensor.matmul(
                        sp, bkv2_sb[:, j, :], ones[:, :TOPK], start=False, stop=True
                    )
                    nc.tensor.matmul(
                        sg, bkv2_sb[:, j, :], ones[:, :NG], start=False, stop=True
                    )
                    # even j: K at rows 0:64, V^T at 64:128; odd j: swapped
                    nc.any.tensor_copy(ktk_sb[hp : hp + 64, dc, :], sp[hp : hp + 64, :])
                    nc.any.tensor_copy(kg_sb[hp : hp + 64, dc, :], sg[hp : hp + 64, :])
                    vq = 64 - hp
                    nc.any.tensor_copy(
                        vtkT_sb[vq : vq + 64, dc, :], sp[vq : vq + 64, :]
                    )
                    nc.any.tensor_copy(
                        vgT_sb[vq : vq + 64, dc, :], sg[vq : vq + 64, :]
                    )

                # ---- transpose V^T staging into [token, dh] layout ----
                for j in range(HPG):
                    vq = 64 - (j % 2) * 64
                    dc = j // 2
                    idsl = id128[vq : vq + 64, vq : vq + 64]
                    for c in range(TOPK // 128):
                        tp = pt_ps.tile([128, 64], BF16, tag="pt")
                        nc.tensor.transpose(
                            tp, vtkT_sb[vq : vq + 64, dc, 128 * c : 128 * (c + 1)],
                            idsl,
                        )
                        nc.any.tensor_copy(vtk_sb[:, c, j, 0:DH], tp)
                    tg = pt_ps.tile([128, 64], BF16, tag="pt")
                    nc.tensor.transpose(tg[0:NG, :], vgT_sb[vq : vq + 64, dc, :NG], idsl)
                    nc.any.tensor_copy(vg_sb[:, j, 0:DH], tg[0:NG, :])

            # ---- per-head attention ----
            with (
                tc.tile_pool(name="attn", bufs=2) as attn,
                tc.tile_pool(name="nrm", bufs=2) as nrm,
                tc.tile_pool(name="st_ps", bufs=2, space="PSUM") as st_ps,
                tc.tile_pool(name="av_ps", bufs=1, space="PSUM") as av_ps,
            ):
                for j in range(HPG):
                    hp = (j % 2) * 64
                    dc = j // 2
                    kTh = kT_sb[hp : hp + 64, dc, :]
                    qTh = qT_sb[hp : hp + 64, dc, :]
                    ET = attn.tile([128, BCH, QB], BF16, tag="ET")
                    for rnd in range(BCH // 2):
                        stp = st_ps.tile([128, 2, QB], F32, tag="st")
                        for i in range(2):
                            cc = 2 * rnd + i
                            nc.tensor.matmul(
                                stp[:, i, :], kTh[:, 128 * cc : 128 * (cc + 1)],
                                qTh, start=True, stop=True,
                            )
                        nc.scalar.activation(
                            ET[:, 2 * rnd : 2 * rnd + 2, :], stp, Exp, scale=SCALE
                        )
                    # top-k scores
                    stk = st_ps.tile([128, 2, QB], F32, tag="st")
                    for c in range(2):
                        nc.tensor.matmul(
                            stk[:, c, :], ktk_sb[hp : hp + 64, dc, 128 * c : 128 * (c + 1)],
                            qTh, start=True, stop=True,
                        )
                    ETk = attn.tile([128, 2, QB], BF16, tag="ETk")
                    nc.scalar.activation(ETk, stk, Exp, scale=SCALE)
                    # global scores
                    stg = st_ps.tile([128, 2, QB], F32, tag="st")
                    nc.tensor.matmul(
                        stg[0:NG, 0, :], kg_sb[hp : hp + 64, dc, :], qTh,
                        start=True, stop=True,
                    )
                    ETg = attn.tile([NG, QB], BF16, tag="ETg")
                    nc.scalar.activation(ETg, stg[0:NG, 0, :], Exp, scale=SCALE)
                    # band mask (in place)
                    nc.vector.tensor_mul(ET, ET, M8_sb)
                    # attention-weighted values; row 64 = branch denominators
                    av = av_ps.tile([DH + 1, 3, QB], F32, tag="av")
                    for tcn in range(BCH):
                        nc.tensor.matmul(
                            av[:, 0, :], v_sb[:, tcn, j, :], ET[:, tcn, :],
                            start=(tcn == 0), stop=(tcn == BCH - 1),
                        )
                    for c in range(TOPK // 128):
                        nc.tensor.matmul(
                            av[:, 1, :], vtk_sb[:, c, j, :], ETk[:, c, :],
                            start=(c == 0), stop=(c == TOPK // 128 - 1),
                        )
                    nc.tensor.matmul(
                        av[:, 2, :], vg_sb[:, j, :], ETg, start=True, stop=True
                    )
                    # normalize: av -> SBUF (frees PSUM), 1/denom on DVE,
                    # DMA the recip row to partition 0 (partition_broadcast
                    # only reads partition 0 on HW), broadcast on gpsimd,
                    # weighted sum on gpsimd. Entirely off the PE.
                    avs = nrm.tile([DH + 1, 3, QB], F32, tag="avs")
                    nc.scalar.copy(avs, av)
                    dn0 = nrm.tile([1, 3, QB], F32, tag="dn0")
                    nc.sync.dma_start(out=dn0, in_=avs[DH : DH + 1, :, :])
                    dbs = nrm.tile([DH, 3, QB], F32, tag="dbs")
                    for b in range(3):
                        nc.gpsimd.partition_broadcast(dbs[:, b, :], dn0[:, b, :])
                    rbs = nrm.tile([DH, 3, QB], F32, tag="rbs")
                    nc.vector.reciprocal_approx_fast(rbs, dbs)
                    ta = nrm.tile([DH, QB], F32, tag="ta")
                    tb = nrm.tile([DH, QB], F32, tag="tb")
                    nc.gpsimd.tensor_mul(ta, avs[0:DH, 0, :], rbs[:, 0, :])
                    nc.gpsimd.tensor_mul(tb, avs[0:DH, 1, :], rbs[:, 1, :])
                    nc.gpsimd.tensor_add(ta, ta, tb)
                    nc.gpsimd.tensor_mul(tb, avs[0:DH, 2, :], rbs[:, 2, :])
                    nc.gpsimd.tensor_add(attnT[:, j, :], ta, tb)
                    if debug and j == 0:
                        nc.sync.dma_start(out=davs[:, :, :], in_=avs)
                        nc.sync.dma_start(out=drin[:, :, :], in_=dn0)
                        nc.sync.dma_start(out=drin0[:, :, :], in_=dn0)
                        nc.sync.dma_start(out=drbs[:, :, :], in_=rbs)
                        dat = nrm.tile([DH, QB], F32, tag="dat")
                        nc.vector.tensor_copy(dat, attnT[:, 0, :])
                        nc.sync.dma_start(out=dattnT[:, :], in_=dat)

            # ---- partial out-projection (Wo/3 folded on host) ----
            with tc.tile_pool(name="yt_ps", bufs=2, space="PSUM") as yt_ps:
                for ddc in range(ECH):
                    yp = yt_ps.tile([128, QB], F32, tag="yt")
                    for j in range(HPG):
                        nc.tensor.matmul(
                            yp, wo_sb[:, j, ddc, :], attnT[:, j, :],
                            start=(j == 0), stop=(j == HPG - 1),
                        )
                    nc.any.tensor_copy(yT_sb[:, ddc, :], yp)
                    nc.sync.dma_start(out=yT[:, ddc, :], in_=yT_sb[:, ddc, :])
    nc.finalize()
    return nc


_PROGS = {}
TRACE = False
LAST_EXEC_NS = {}


def _get_progs():
    if "a" not in _PROGS:
        _PROGS["a"] = _build_phase_a()
        _PROGS["b"] = _build_phase_b()
    return _PROGS["a"], _PROGS["b"]


def _pm(arr, dtype):
    """[768, T] -> partition-major [128, 6, T] contiguous."""
    d, t = arr.shape
    assert d == D
    return np.ascontiguousarray(
        arr.reshape(ECH, 128, t).transpose(1, 0, 2).astype(dtype)
    )


def _inputs_b(inputs, xT, topk_idx):
    """Build the 8 per-core phase-B input maps."""
    bf = ml_dtypes.bfloat16
    WqT = inputs["Wq"].T.astype(np.float32)
    WkT = inputs["Wk"].T.astype(np.float32)
    WvT = inputs["Wv"].T.astype(np.float32)
    WoT3 = (inputs["Wo"].T / 3.0).astype(np.float32)      # [768 in, 768 out]
    bq, bk, bv = (np.asarray(inputs[k], np.float32) for k in ("bq", "bk", "bv"))

    grp = []
    for g in range(HG):
        hs = slice(HPG * g * DH, HPG * (g + 1) * DH)
        wq, wk, wv = WqT[:, hs], WkT[:, hs], WvT[:, hs]   # [768, 384]
        Wkv2 = np.empty((D, HPG, 128), np.float32)
        bkv2 = np.empty((1, HPG, 128), np.float32)
        for j in range(HPG):
            js = slice(j * DH, (j + 1) * DH)
            if j % 2 == 0:
                Wkv2[:, j, 0:64], Wkv2[:, j, 64:128] = wk[:, js], wv[:, js]
                bkv2[0, j, 0:64], bkv2[0, j, 64:128] = bk[hs][js], bv[hs][js]
            else:
                Wkv2[:, j, 0:64], Wkv2[:, j, 64:128] = wv[:, js], wk[:, js]
                bkv2[0, j, 0:64], bkv2[0, j, 64:128] = bv[hs][js], bk[hs][js]
        wo = np.empty((DH, HPG, ECH, 128), np.float32)
        for j in range(HPG):
            for dcc in range(ECH):
                wo[:, j, dcc, :] = WoT3[
                    HPG * g * DH + j * DH : HPG * g * DH + (j + 1) * DH,
                    128 * dcc : 128 * (dcc + 1),
                ]
        xk = np.concatenate(
            [xT[:, topk_idx[HPG * g + j]] for j in range(HPG)], axis=1
        )
        grp.append(
            dict(
                WqTg=_pm(np.ascontiguousarray(wq), bf),
                WkTg=_pm(np.ascontiguousarray(wk), bf),
                WvTg=_pm(np.ascontiguousarray(wv), bf),
                bqg=np.ascontiguousarray(bq[hs][None, :]).astype(bf),
                bkg=np.ascontiguousarray(bk[hs][None, :]).astype(bf),
                bvg=np.ascontiguousarray(bv[hs][None, :]).astype(bf),
                Wkv2=_pm(Wkv2.reshape(D, HPG * 128), bf).reshape(128, ECH, HPG, 128),
                bkv2=bkv2.astype(bf),
                Wo3=np.ascontiguousarray(wo.astype(bf)),
                xTk=_pm(xk, bf),
            )
        )

    xTg_pm = _pm(xT[:, :NG], bf)
    p = np.arange(128)[:, None, None]
    kk = np.arange(BCH)[None, :, None]
    sl = np.arange(QB)[None, None, :]
    in_b = []
    for c in range(NCORES):
        qb, g = c // HG, c % HG
        s0 = min(max(4 * qb - 2, 0), S // 128 - BCH)
        M8 = (np.abs(128 * s0 + 128 * kk + p - (QB * qb + sl)) <= LWH).astype(bf)
        in_b.append(
            dict(
                grp[g],
                xTb=_pm(xT[:, 128 * s0 : 128 * s0 + BT], bf),
                xTq=_pm(xT[:, QB * qb : QB * (qb + 1)], bf),
                xTg=xTg_pm,
                M8=np.ascontiguousarray(M8),
            )
        )
    return in_b


def kernel(**inputs):
    x = np.asarray(inputs["x"][0], np.float32)            # [S, D]
    xT = np.ascontiguousarray(x.T)                        # [D, S]
    nc_a, nc_b = _get_progs()

    # ---- phase A: token scores, sharded over 8 cores ----
    WsT_pm = _pm(np.ascontiguousarray(inputs["Ws"].T, np.float32), np.float32)
    bs_row = np.ascontiguousarray(inputs["bs"][None, :], np.float32)
    TPC = S // NCORES
    in_a = [
        {
            "xTa": _pm(xT[:, TPC * c : TPC * (c + 1)], np.float32),
            "WsT": WsT_pm,
            "bs_row": bs_row,
        }
        for c in range(NCORES)
    ]
    ra = run_bass_kernel_spmd(nc_a, in_a, list(range(NCORES)), trace=TRACE)
    ts = np.concatenate([r["ts"] for r in ra.results], axis=1)  # [H, S]
    LAST_EXEC_NS["phase_a"] = ra.exec_time_ns

    topk_idx = [np.argpartition(-ts[h], TOPK)[:TOPK] for h in range(H)]
    in_b = _inputs_b(inputs, xT, topk_idx)
    res = run_bass_kernel_spmd(nc_b, in_b, list(range(NCORES)), trace=TRACE)
    LAST_EXEC_NS["phase_b"] = res.exec_time_ns

    bo = np.asarray(inputs["bo"], np.float32)
    out = np.empty((S, D), np.float32)
    for qb in range(NQB):
        ypm = res.results[2 * qb]["yT"] + res.results[2 * qb + 1]["yT"]
        yfull = ypm.transpose(1, 0, 2).reshape(D, QB)     # [768, 512]
        out[QB * qb : QB * (qb + 1)] = yfull.T + bo
    return out.reshape(1, S, D)


# revision 24
# speedup vs baseline: 2.3638x; 1.4441x over previous
# Trainium2 Bass kernel for DeepSeek-style sparse attention.
# Self-contained: hardcodes shapes from the problem spec.
#   x [1, 2048, 768]; Wq/Wk/Wv/Wo [768, 768]; biases [768]; Ws [12, 768]; bs [12]
#
# Sharding: 8 cores = 4 query-blocks (512 queries) x 2 head-groups (6 heads).
# Each core computes, for its 512 queries and 6 heads:
#   - band K/Q/V projections over an 8-chunk (1024-token) window that covers
#     the +-256 local band of its query block (host slices x accordingly),
#   - a compact top-k branch from host-gathered x columns (256 per head,
#     indices from a tiny fp32 phase-A token-score kernel + host argpartition),
#   - the 16-token global branch,
#   - a partial out-projection over its 6 heads' dims. The host sums the two
#     head-group partials per query block and adds bo (no device collective).
# Everything post-PSUM runs in bf16; phase A stays fp32 so the top-k
# selection matches the fp32 reference bit-for-bit on near-ties.
import sys
import numpy as np
import ml_dtypes

sys.path.insert(0, "/opt/trn_rl_repo")

import concourse.bass as bass
from concourse import bacc
import concourse.mybir as mybir
from concourse.tile import TileContext
from concourse.bass_utils import run_bass_kernel_spmd
from concourse.masks import make_identity

S = 2048
D = 768
H = 12
DH = 64
NCORES = 8
NQB = 4                    # query blocks
QB = S // NQB              # 512 queries per block
HG = 2                     # head groups
HPG = H // HG              # 6 heads per group
ECH = D // 128             # 6 embedding chunks
BCH = 8                    # band chunks per core (8 x 128 = 1024 tokens)
BT = BCH * 128             # 1024 band tokens
TOPK = 256
NG = 16
LWH = 256                  # local window half-width
SCALE = 1.0 / np.sqrt(DH)
F32 = mybir.dt.float32
BF16 = mybir.dt.bfloat16
Exp = mybir.ActivationFunctionType.Exp


def _patch_tile_drain():
    """This walrus build rejects sem-waits on Drain instructions ("Too many
    sync wait commands"). Emit the tail waits as individual SemWait ops on
    the sync engine instead, then a bare drain."""
    if getattr(TileContext, "_drain_patched", False):
        return

    def _drain_and_barrier(self, tick_clock, wait_clock):
        nc = self.nc
        clock = tick_clock.global_clock
        for proc, handle in sorted(self.sems.allocated().items()):
            tick = clock[proc]
            if tick <= 0:
                continue
            mult = 16 if "DMA" in handle.name else 1
            nc.sync.wait_ge(handle, tick * mult)
        nc.sync.drain()
        nc.all_engine_barrier()
        popped = nc._tile_sem_poison_stack.pop()
        assert popped is self._sem_poison
        nc.clear_and_free_semaphores(list(self.sems.allocated().values()))
        nc.all_engine_barrier()

    TileContext._drain_and_barrier = _drain_and_barrier
    TileContext._drain_patched = True


def _build_phase_a():
    """ts[h, 256] = (Ws @ x^T + bs) for this core's 256-token slice, fp32."""
    _patch_tile_drain()
    nc = bacc.Bacc()
    TPC = S // NCORES  # 256 tokens per core
    xTa = nc.declare_dram_parameter("xTa", [128, ECH, TPC], F32, isOutput=False)
    WsT = nc.declare_dram_parameter("WsT", [128, ECH, H], F32, isOutput=False)
    bs_row = nc.declare_dram_parameter("bs_row", [1, H], F32, isOutput=False)
    ts = nc.declare_dram_parameter("ts", [H, TPC], F32, isOutput=True)

    with TileContext(nc) as tc:
        with (
            tc.tile_pool(name="sb", bufs=1) as sb,
            tc.tile_pool(name="ps", bufs=1, space="PSUM") as ps,
        ):
            xTa_sb = sb.tile([128, ECH, TPC], F32)
            WsT_sb = sb.tile([128, ECH, H], F32)
            bs_sb = sb.tile([1, H], F32)
            ones = sb.tile([1, TPC], F32)
            nc.vector.memset(ones, 1.0)
            nc.sync.dma_start(out=bs_sb, in_=bs_row[:, :])
            nc.sync.dma_start(out=WsT_sb, in_=WsT[:, :, :])
            nc.sync.dma_start(out=xTa_sb, in_=xTa[:, :, :])
            acc = ps.tile([H, TPC], F32)
            for ec in range(ECH):
                nc.tensor.matmul(
                    acc, WsT_sb[:, ec, :], xTa_sb[:, ec, :],
                    start=(ec == 0), stop=False,
                )
            nc.tensor.matmul(acc, bs_sb, ones, start=False, stop=True)
            ts_sb = sb.tile([H, TPC], F32)
            nc.vector.tensor_copy(ts_sb, acc)
            nc.sync.dma_start(out=ts[:, :], in_=ts_sb)
    nc.finalize()
    return nc


def _build_phase_b(debug=False):
    """Per-core sparse attention for 512 queries x 6 heads (see header)."""
    _patch_tile_drain()
    nc = bacc.Bacc()
    if debug:
        davs = nc.declare_dram_parameter("davs", [DH + 1, 3, QB], F32, isOutput=True)
        drin = nc.declare_dram_parameter("drin", [1, 3, QB], F32, isOutput=True)
        drin0 = nc.declare_dram_parameter("drin0", [1, 3, QB], F32, isOutput=True)
        drbs = nc.declare_dram_parameter("drbs", [DH, 3, QB], F32, isOutput=True)
        dattnT = nc.declare_dram_parameter("dattnT", [DH, QB], F32, isOutput=True)
    # host-prepared, partition-major, bf16
    xTb = nc.declare_dram_parameter("xTb", [128, ECH, BT], BF16, isOutput=False)
    xTq = nc.declare_dram_parameter("xTq", [128, ECH, QB], BF16, isOutput=False)
    xTk = nc.declare_dram_parameter("xTk", [128, ECH, HPG * TOPK], BF16, isOutput=False)
    xTg = nc.declare_dram_parameter("xTg", [128, ECH, NG], BF16, isOutput=False)
    # per-type weight slices for this head group (384 = 6 heads x 64 dims)
    WqTg = nc.declare_dram_parameter("WqTg", [128, ECH, 384], BF16, isOutput=False)
    WkTg = nc.declare_dram_parameter("WkTg", [128, ECH, 384], BF16, isOutput=False)
    WvTg = nc.declare_dram_parameter("WvTg", [128, ECH, 384], BF16, isOutput=False)
    bqg = nc.declare_dram_parameter("bqg", [1, 384], BF16, isOutput=False)
    bkg = nc.declare_dram_parameter("bkg", [1, 384], BF16, isOutput=False)
    bvg = nc.declare_dram_parameter("bvg", [1, 384], BF16, isOutput=False)
    # stacked [Wk_h | Wv_h] (even local head) / [Wv_h | Wk_h] (odd) for the
    # compact top-k / global projections
    Wkv2 = nc.declare_dram_parameter("Wkv2", [128, ECH, HPG, 128], BF16, isOutput=False)
    bkv2 = nc.declare_dram_parameter("bkv2", [1, HPG, 128], BF16, isOutput=False)
    # Wo^T/3 slices: wo[dh, j, dc, :] over this group's heads
    Wo3 = nc.declare_dram_parameter("Wo3", [DH, HPG, ECH, 128], BF16, isOutput=False)
    M8 = nc.declare_dram_parameter("M8", [128, BCH, QB], BF16, isOutput=False)
    yT = nc.declare_dram_parameter("yT", [128, ECH, QB], F32, isOutput=True)

    with TileContext(nc) as tc, nc.allow_low_precision(reason="bf16 validated vs reference"):
        with tc.tile_pool(name="perm", bufs=1) as perm:
            kT_sb = perm.tile([128, HPG // 2, BT], BF16)       # 2 heads / 128 parts
            qT_sb = perm.tile([128, HPG // 2, QB], BF16)
            ktk_sb = perm.tile([128, HPG // 2, TOPK], BF16)
            vtkT_sb = perm.tile([128, HPG // 2, TOPK], BF16)   # staging (pre-transpose)
            vtk_sb = perm.tile([128, TOPK // 128, HPG, DH + 1], BF16)
            kg_sb = perm.tile([128, HPG // 2, NG], BF16)
            vgT_sb = perm.tile([128, HPG // 2, NG], BF16)      # staging
            vg_sb = perm.tile([NG, HPG, DH + 1], BF16)
            v_sb = perm.tile([128, BCH, HPG, DH + 1], BF16)
            M8_sb = perm.tile([128, BCH, QB], BF16)
            attnT = perm.tile([DH, HPG, QB], BF16)
            yT_sb = perm.tile([128, ECH, QB], F32)
            wo_sb = perm.tile([DH, HPG, ECH, 128], BF16)
            bqg_sb = perm.tile([1, 384], BF16)
            bkg_sb = perm.tile([1, 384], BF16)
            bvg_sb = perm.tile([1, 384], BF16)
            bkv2_sb = perm.tile([1, HPG, 128], BF16)
            ones = perm.tile([1, QB], BF16)
            id128 = perm.tile([128, 128], BF16)
            nc.vector.memset(ones, 1.0)
            make_identity(nc, id128)
            nc.vector.memset(v_sb[:, :, :, DH : DH + 1], 1.0)
            nc.vector.memset(vtk_sb[:, :, :, DH : DH + 1], 1.0)
            nc.vector.memset(vg_sb[:, :, DH : DH + 1], 1.0)
            nc.sync.dma_start(out=M8_sb, in_=M8[:, :, :])
            nc.sync.dma_start(out=bqg_sb, in_=bqg[:, :])
            nc.sync.dma_start(out=bkg_sb, in_=bkg[:, :])
            nc.sync.dma_start(out=bvg_sb, in_=bvg[:, :])
            nc.sync.dma_start(out=bkv2_sb, in_=bkv2[:, :, :])
            nc.sync.dma_start(out=wo_sb, in_=Wo3[:, :, :, :])

            with (
                tc.tile_pool(name="xin", bufs=1) as xin,
                tc.tile_pool(name="pj_ps", bufs=4, space="PSUM") as pj_ps,
                tc.tile_pool(name="pg_ps", bufs=2, space="PSUM") as pg_ps,
                tc.tile_pool(name="pt_ps", bufs=2, space="PSUM") as pt_ps,
            ):
                xTb_sb = xin.tile([128, ECH, BT], BF16)
                xTq_sb = xin.tile([128, ECH, QB], BF16)
                xTk_sb = xin.tile([128, ECH, HPG * TOPK], BF16)
                xTg_sb = xin.tile([128, ECH, NG], BF16)
                WqTg_sb = xin.tile([128, ECH, 384], BF16)
                WkTg_sb = xin.tile([128, ECH, 384], BF16)
                WvTg_sb = xin.tile([128, ECH, 384], BF16)
                Wkv2_sb = xin.tile([128, ECH, HPG, 128], BF16)
                for ec in range(ECH):
                    nc.sync.dma_start(out=WkTg_sb[:, ec, :], in_=WkTg[:, ec])
                    nc.sync.dma_start(out=WqTg_sb[:, ec, :], in_=WqTg[:, ec])
                    nc.sync.dma_start(out=WvTg_sb[:, ec, :], in_=WvTg[:, ec])
                    nc.sync.dma_start(out=xTb_sb[:, ec, :], in_=xTb[:, ec])
                for ec in range(ECH):
                    nc.sync.dma_start(out=xTq_sb[:, ec, :], in_=xTq[:, ec])
                    nc.sync.dma_start(out=xTk_sb[:, ec, :], in_=xTk[:, ec])
                    nc.sync.dma_start(out=Wkv2_sb[:, ec], in_=Wkv2[:, ec])
                nc.sync.dma_start(out=xTg_sb, in_=xTg[:, :, :])

                # ---- band K^T [dims, 1024] and Q^T [dims, 512] ----
                for dc in range(HPG // 2):
                    ds = slice(128 * dc, 128 * (dc + 1))
                    for half in range(2):
                        t0 = 512 * half
                        kp = pj_ps.tile([128, 512], F32, tag="pj")
                        for ec in range(ECH):
                            nc.tensor.matmul(
                                kp, WkTg_sb[:, ec, ds],
                                xTb_sb[:, ec, t0 : t0 + 512],
                                start=(ec == 0), stop=False,
                            )
                        nc.tensor.matmul(
                            kp, bkg_sb[:, ds], ones, start=False, stop=True
                        )
                        nc.any.tensor_copy(kT_sb[:, dc, t0 : t0 + 512], kp)
                    qp = pj_ps.tile([128, 512], F32, tag="pj")
                    for ec in range(ECH):
                        nc.tensor.matmul(
                            qp, WqTg_sb[:, ec, ds], xTq_sb[:, ec, :],
                            start=(ec == 0), stop=False,
                        )
                    nc.tensor.matmul(
                        qp, bqg_sb[:, ds], ones, start=False, stop=True
                    )
                    nc.any.tensor_copy(qT_sb[:, dc, :], qp)

                # ---- band V [1024 tokens, 6 heads x 64] ----
                for tcn in range(BCH):
                    t0 = 128 * tcn
                    vp = pj_ps.tile([128, HPG * DH], F32, tag="pj")
                    for ec in range(ECH):
                        nc.tensor.matmul(
                            vp, xTb_sb[:, ec, t0 : t0 + 128],
                            WvTg_sb[:, ec, :],
                            start=(ec == 0), stop=False,
                        )
                    nc.tensor.matmul(
                        vp, ones[:, :128], bvg_sb,
                        start=False, stop=True,
                    )
                    nc.any.tensor_copy(
                        v_sb[:, tcn, :, 0:DH],
                        vp.rearrange("p (h d) -> p h d", d=DH),
                    )

                # ---- compact top-k + global K/V per head (stacked K|V) ----
                for j in range(HPG):
                    hp = (j % 2) * 64
                    dc = j // 2
                    sp = pj_ps.tile([128, TOPK], F32, tag="pj")
                    sg = pg_ps.tile([128, NG], F32, tag="pg")
                    for ec in range(ECH):
                        nc.tensor.matmul(
                            sp, Wkv2_sb[:, ec, j, :],
                            xTk_sb[:, ec, TOPK * j : TOPK * (j + 1)],
                            start=(ec == 0), stop=False,
                        )
                        nc.tensor.matmul(
                            sg, Wkv2_sb[:, ec, j, :], xTg_sb[:, ec, :],
                            start=(ec == 0), stop=False,
                        )
                    nc.tensor.matmul(
                        sp, bkv2_sb[:, j, :], ones[:, :TOPK], start=False, stop=True
                    )
                    nc.tensor.matmul(
                        sg, bkv2_sb[:, j, :], ones[:, :NG], start=False, stop=True
                    )
                    # even j: K at rows 0:64, V^T at 64:128; odd j: swapped
                    nc.any.tensor_copy(ktk_sb[hp : hp + 64, dc, :], sp[hp : hp + 64, :])
                    nc.any.tensor_copy(kg_sb[hp : hp + 64, dc, :], sg[hp : hp + 64, :])
                    vq = 64 - hp
                    nc.any.tensor_copy(
                        vtkT_sb[vq : vq + 64, dc, :], sp[vq : vq + 64, :]
                    )
                    nc.any.tensor_copy(
                        vgT_sb[vq : vq + 64, dc, :], sg[vq : vq + 64, :]
                    )

                # ---- transpose V^T staging into [token, dh] layout ----
                for j in range(HPG):
                    vq = 64 - (j % 2) * 64
                    dc = j // 2
                    idsl = id128[vq : vq + 64, vq : vq + 64]
                    for c in range(TOPK // 128):
                        tp = pt_ps.tile([128, 64], BF16, tag="pt")
                        nc.tensor.transpose(
                            tp, vtkT_sb[vq : vq + 64, dc, 128 * c : 128 * (c + 1)],
                            idsl,
                        )
                        nc.any.tensor_copy(vtk_sb[:, c, j, 0:DH], tp)
                    tg = pt_ps.tile([128, 64], BF16, tag="pt")
                    nc.tensor.transpose(tg[0:NG, :], vgT_sb[vq : vq + 64, dc, :NG], idsl)
                    nc.any.tensor_copy(vg_sb[:, j, 0:DH], tg[0:NG, :])

            # ---- per-head attention ----
            with (
                tc.tile_pool(name="attn", bufs=2) as attn,
                tc.tile_pool(name="nrm", bufs=2) as nrm,
                tc.tile_pool(name="st_ps", bufs=2, space="PSUM") as st_ps,
                tc.tile_pool(name="av_ps", bufs=1, space="PSUM") as av_ps,
            ):
                for j in range(HPG):
                    hp = (j % 2) * 64
                    dc = j // 2
                    kTh = kT_sb[hp : hp + 64, dc, :]
                    qTh = qT_sb[hp : hp + 64, dc, :]
                    ET = attn.tile([128, BCH, QB], BF16, tag="ET")
                    for rnd in range(BCH // 2):
                        stp = st_ps.tile([128, 2, QB], F32, tag="st")
                        for i in range(2):
                            cc = 2 * rnd + i
                            nc.tensor.matmul(
                                stp[:, i, :], kTh[:, 128 * cc : 128 * (cc + 1)],
                                qTh, start=True, stop=True,
                            )
                        nc.scalar.activation(
                            ET[:, 2 * rnd : 2 * rnd + 2, :], stp, Exp, scale=SCALE
                        )
                    # top-k scores
                    stk = st_ps.tile([128, 2, QB], F32, tag="st")
                    for c in range(2):
                        nc.tensor.matmul(
                            stk[:, c, :], ktk_sb[hp : hp + 64, dc, 128 * c : 128 * (c + 1)],
                            qTh, start=True, stop=True,
                        )
                    ETk = attn.tile([128, 2, QB], BF16, tag="ETk")
                    nc.scalar.activation(ETk, stk, Exp, scale=SCALE)
                    # global scores
                    stg = st_ps.tile([128, 2, QB], F32, tag="st")
                    nc.tensor.matmul(
                        stg[0:NG, 0, :], kg_sb[hp : hp + 64, dc, :], qTh,
                        start=True, stop=True,
                    )
                    ETg = attn.tile([NG, QB], BF16, tag="ETg")
                    nc.scalar.activation(ETg, stg[0:NG, 0, :], Exp, scale=SCALE)
                    # band mask (in place)
                    nc.vector.tensor_mul(ET, ET, M8_sb)
                    # attention-weighted values; row 64 = branch denominators
                    av = av_ps.tile([DH + 1, 3, QB], F32, tag="av")
                    for tcn in range(BCH):
                        nc.tensor.matmul(
                            av[:, 0, :], v_sb[:, tcn, j, :], ET[:, tcn, :],
                            start=(tcn == 0), stop=(tcn == BCH - 1),
                        )
                    for c in range(TOPK // 128):
                        nc.tensor.matmul(
                            av[:, 1, :], vtk_sb[:, c, j, :], ETk[:, c, :],
                            start=(c == 0), stop=(c == TOPK // 128 - 1),
                        )
                    nc.tensor.matmul(
                        av[:, 2, :], vg_sb[:, j, :], ETg, start=True, stop=True
                    )
                    # normalize: av -> SBUF (frees PSUM), 1/denom on DVE,
                    # DMA the recip row to partition 0 (partition_broadcast
                    # only reads partition 0 on HW), broadcast on gpsimd,
                    # weighted sum on gpsimd. Entirely off the PE.
                    avs = nrm.tile([DH + 1, 3, QB], F32, tag="avs")
                    nc.scalar.copy(avs, av)
                    dn0 = nrm.tile([1, 3, QB], F32, tag="dn0")
                    nc.sync.dma_start(out=dn0, in_=avs[DH : DH + 1, :, :])
                    dbs = nrm.tile([DH, 3, QB], F32, tag="dbs")
                    for b in range(3):
                        nc.gpsimd.partition_broadcast(dbs[:, b, :], dn0[:, b, :])
                    rbs = nrm.tile([DH, 3, QB], F32, tag="rbs")
                    nc.vector.reciprocal_approx_fast(rbs, dbs)
                    ta = nrm.tile([DH, QB], F32, tag="ta")
                    tb = nrm.tile([DH, QB], F32, tag="tb")
                    nc.vector.tensor_mul(ta, avs[0:DH, 0, :], rbs[:, 0, :])
                    nc.vector.tensor_mul(tb, avs[0:DH, 1, :], rbs[:, 1, :])
                    nc.vector.tensor_add(ta, ta, tb)
                    nc.vector.tensor_mul(tb, avs[0:DH, 2, :], rbs[:, 2, :])
                    nc.vector.tensor_add(attnT[:, j, :], ta, tb)
                    if debug and j == 0:
                        nc.sync.dma_start(out=davs[:, :, :], in_=avs)
                        nc.sync.dma_start(out=drin[:, :, :], in_=dn0)
                        nc.sync.dma_start(out=drin0[:, :, :], in_=dn0)
                        nc.sync.dma_start(out=drbs[:, :, :], in_=rbs)
                        dat = nrm.tile([DH, QB], F32, tag="dat")
                        nc.vector.tensor_copy(dat, attnT[:, 0, :])
                        nc.sync.dma_start(out=dattnT[:, :], in_=dat)

            # ---- partial out-projection (Wo/3 folded on host) ----
            with tc.tile_pool(name="yt_ps", bufs=2, space="PSUM") as yt_ps:
                for ddc in range(ECH):
                    yp = yt_ps.tile([128, QB], F32, tag="yt")
                    for j in range(HPG):
                        nc.tensor.matmul(
                            yp, wo_sb[:, j, ddc, :], attnT[:, j, :],
                            start=(j == 0), stop=(j == HPG - 1),
                        )
                    nc.any.tensor_copy(yT_sb[:, ddc, :], yp)
                    nc.sync.dma_start(out=yT[:, ddc, :], in_=yT_sb[:, ddc, :])
    nc.finalize()
    return nc


_PROGS = {}
TRACE = False
LAST_EXEC_NS = {}


def _get_progs():
    if "a" not in _PROGS:
        _PROGS["a"] = _build_phase_a()
        _PROGS["b"] = _build_phase_b()
    return _PROGS["a"], _PROGS["b"]


def _pm(arr, dtype):
    """[768, T] -> partition-major [128, 6, T] contiguous."""
    d, t = arr.shape
    assert d == D
    return np.ascontiguousarray(
        arr.reshape(ECH, 128, t).transpose(1, 0, 2).astype(dtype)
    )


def _inputs_b(inputs, xT, topk_idx):
    """Build the 8 per-core phase-B input maps."""
    bf = ml_dtypes.bfloat16
    WqT = inputs["Wq"].T.astype(np.float32)
    WkT = inputs["Wk"].T.astype(np.float32)
    WvT = inputs["Wv"].T.astype(np.float32)
    WoT3 = (inputs["Wo"].T / 3.0).astype(np.float32)      # [768 in, 768 out]
    bq, bk, bv = (np.asarray(inputs[k], np.float32) for k in ("bq", "bk", "bv"))

    grp = []
    for g in range(HG):
        hs = slice(HPG * g * DH, HPG * (g + 1) * DH)
        wq, wk, wv = WqT[:, hs], WkT[:, hs], WvT[:, hs]   # [768, 384]
        Wkv2 = np.empty((D, HPG, 128), np.float32)
        bkv2 = np.empty((1, HPG, 128), np.float32)
        for j in range(HPG):
            js = slice(j * DH, (j + 1) * DH)
            if j % 2 == 0:
                Wkv2[:, j, 0:64], Wkv2[:, j, 64:128] = wk[:, js], wv[:, js]
                bkv2[0, j, 0:64], bkv2[0, j, 64:128] = bk[hs][js], bv[hs][js]
            else:
                Wkv2[:, j, 0:64], Wkv2[:, j, 64:128] = wv[:, js], wk[:, js]
                bkv2[0, j, 0:64], bkv2[0, j, 64:128] = bv[hs][js], bk[hs][js]
        wo = np.empty((DH, HPG, ECH, 128), np.float32)
        for j in range(HPG):
            for dcc in range(ECH):
                wo[:, j, dcc, :] = WoT3[
                    HPG * g * DH + j * DH : HPG * g * DH + (j + 1) * DH,
                    128 * dcc : 128 * (dcc + 1),
                ]
        xk = np.concatenate(
            [xT[:, topk_idx[HPG * g + j]] for j in range(HPG)], axis=1
        )
        grp.append(
            dict(
                WqTg=_pm(np.ascontiguousarray(wq), bf),
                WkTg=_pm(np.ascontiguousarray(wk), bf),
                WvTg=_pm(np.ascontiguousarray(wv), bf),
                bqg=np.ascontiguousarray(bq[hs][None, :]).astype(bf),
                bkg=np.ascontiguousarray(bk[hs][None, :]).astype(bf),
                bvg=np.ascontiguousarray(bv[hs][None, :]).astype(bf),
                Wkv2=_pm(Wkv2.reshape(D, HPG * 128), bf).reshape(128, ECH, HPG, 128),
                bkv2=bkv2.astype(bf),
                Wo3=np.ascontiguousarray(wo.astype(bf)),
                xTk=_pm(xk, bf),
            )
        )

    xTg_pm = _pm(xT[:, :NG], bf)
    p = np.arange(128)[:, None, None]
    kk = np.arange(BCH)[None, :, None]
    sl = np.arange(QB)[None, None, :]
    in_b = []
    for c in range(NCORES):
        qb, g = c // HG, c % HG
        s0 = min(max(4 * qb - 2, 0), S // 128 - BCH)
        M8 = (np.abs(128 * s0 + 128 * kk + p - (QB * qb + sl)) <= LWH).astype(bf)
        in_b.append(
            dict(
                grp[g],
                xTb=_pm(xT[:, 128 * s0 : 128 * s0 + BT], bf),
                xTq=_pm(xT[:, QB * qb : QB * (qb + 1)], bf),
                xTg=xTg_pm,
                M8=np.ascontiguousarray(M8),
            )
        )
    return in_b


def kernel(**inputs):
    x = np.asarray(inputs["x"][0], np.float32)            # [S, D]
    xT = np.ascontiguousarray(x.T)                        # [D, S]
    nc_a, nc_b = _get_progs()

    # ---- phase A: token scores, sharded over 8 cores ----
    WsT_pm = _pm(np.ascontiguousarray(inputs["Ws"].T, np.float32), np.float32)
    bs_row = np.ascontiguousarray(inputs["bs"][None, :], np.float32)
    TPC = S // NCORES
    in_a = [
        {
            "xTa": _pm(xT[:, TPC * c : TPC * (c + 1)], np.float32),
            "WsT": WsT_pm,
            "bs_row": bs_row,
        }
        for c in range(NCORES)
    ]
    ra = run_bass_kernel_spmd(nc_a, in_a, list(range(NCORES)), trace=TRACE)
    ts = np.concatenate([r["ts"] for r in ra.results], axis=1)  # [H, S]
    LAST_EXEC_NS["phase_a"] = ra.exec_time_ns

    topk_idx = [np.argpartition(-ts[h], TOPK)[:TOPK] for h in range(H)]
    in_b = _inputs_b(inputs, xT, topk_idx)
    res = run_bass_kernel_spmd(nc_b, in_b, list(range(NCORES)), trace=TRACE)
    LAST_EXEC_NS["phase_b"] = res.exec_time_ns

    bo = np.asarray(inputs["bo"], np.float32)
    out = np.empty((S, D), np.float32)
    for qb in range(NQB):
        ypm = res.results[2 * qb]["yT"] + res.results[2 * qb + 1]["yT"]
        yfull = ypm.transpose(1, 0, 2).reshape(D, QB)     # [768, 512]
        out[QB * qb : QB * (qb + 1)] = yfull.T + bo
    return out.reshape(1, S, D)


# revision 32
# speedup vs baseline: 2.4353x; 1.0303x over previous
# Trainium2 Bass kernel for DeepSeek-style sparse attention.
# Self-contained: hardcodes shapes from the problem spec.
#   x [1, 2048, 768]; Wq/Wk/Wv/Wo [768, 768]; biases [768]; Ws [12, 768]; bs [12]
#
# Sharding: 8 cores = 4 query-blocks (512 queries) x 2 head-groups (6 heads).
# Each core computes, for its 512 queries and 6 heads:
#   - band K/Q/V projections over an 8-chunk (1024-token) window that covers
#     the +-256 local band of its query block (host slices x accordingly),
#   - a compact top-k branch from host-gathered x columns (256 per head,
#     indices from a tiny fp32 phase-A token-score kernel + host argpartition),
#   - the 16-token global branch,
#   - a partial out-projection over its 6 heads' dims. The host sums the two
#     head-group partials per query block and adds bo (no device collective).
# Everything post-PSUM runs in bf16; phase A stays fp32 so the top-k
# selection matches the fp32 reference bit-for-bit on near-ties.
import sys
import numpy as np
import ml_dtypes

sys.path.insert(0, "/opt/trn_rl_repo")

import concourse.bass as bass
from concourse import bacc
import concourse.mybir as mybir
from concourse.tile import TileContext
from concourse.bass_utils import run_bass_kernel_spmd
from concourse.masks import make_identity

S = 2048
D = 768
H = 12
DH = 64
NCORES = 8
NQB = 4                    # query blocks
QB = S // NQB              # 512 queries per block
HG = 2                     # head groups
HPG = H // HG              # 6 heads per group
ECH = D // 128             # 6 embedding chunks
BCH = 8                    # band chunks per core (8 x 128 = 1024 tokens)
BT = BCH * 128             # 1024 band tokens
TOPK = 256
NG = 16
LWH = 256                  # local window half-width
SCALE = 1.0 / np.sqrt(DH)
F32 = mybir.dt.float32
BF16 = mybir.dt.bfloat16
Exp = mybir.ActivationFunctionType.Exp


def _patch_tile_drain():
    """This walrus build rejects sem-waits on Drain instructions ("Too many
    sync wait commands"). Emit the tail waits as individual SemWait ops on
    the sync engine instead, then a bare drain."""
    if getattr(TileContext, "_drain_patched", False):
        return

    def _drain_and_barrier(self, tick_clock, wait_clock):
        nc = self.nc
        clock = tick_clock.global_clock
        for proc, handle in sorted(self.sems.allocated().items()):
            tick = clock[proc]
            if tick <= 0:
                continue
            mult = 16 if "DMA" in handle.name else 1
            nc.sync.wait_ge(handle, tick * mult)
        nc.sync.drain()
        nc.all_engine_barrier()
        popped = nc._tile_sem_poison_stack.pop()
        assert popped is self._sem_poison
        nc.clear_and_free_semaphores(list(self.sems.allocated().values()))
        nc.all_engine_barrier()

    TileContext._drain_and_barrier = _drain_and_barrier
    TileContext._drain_patched = True


def _build_phase_a():
    """ts[h, 256] = (Ws @ x^T + bs) for this core's 256-token slice.

    Runs in f32r (TF32-ish); the host re-evaluates tokens near the top-k
    threshold in exact fp32, so rounding cannot flip the selection."""
    _patch_tile_drain()
    nc = bacc.Bacc()
    F32R = mybir.dt.float32r
    TPC = S // NCORES  # 256 tokens per core
    xTa = nc.declare_dram_parameter("xTa", [128, ECH, TPC], F32R, isOutput=False)
    WsT = nc.declare_dram_parameter("WsT", [128, ECH, H], F32R, isOutput=False)
    bs_row = nc.declare_dram_parameter("bs_row", [1, H], F32R, isOutput=False)
    ts = nc.declare_dram_parameter("ts", [H, TPC], F32, isOutput=True)

    with TileContext(nc) as tc, nc.allow_low_precision(reason="host refines boundary"):
        with (
            tc.tile_pool(name="sb", bufs=1) as sb,
            tc.tile_pool(name="ps", bufs=1, space="PSUM") as ps,
        ):
            xTa_sb = sb.tile([128, ECH, TPC], F32R)
            WsT_sb = sb.tile([128, ECH, H], F32R)
            bs_sb = sb.tile([1, H], F32R)
            ones = sb.tile([1, TPC], F32R)
            onesf = sb.tile([1, TPC], F32)
            nc.vector.memset(onesf, 1.0)
            nc.vector.tensor_copy(ones, onesf)
            nc.sync.dma_start(out=bs_sb, in_=bs_row[:, :])
            nc.sync.dma_start(out=WsT_sb, in_=WsT[:, :, :])
            for ec in range(ECH):
                nc.sync.dma_start(out=xTa_sb[:, ec, :], in_=xTa[:, ec])
            acc = ps.tile([H, TPC], F32)
            for ec in range(ECH):
                nc.tensor.matmul(
                    acc, WsT_sb[:, ec, :], xTa_sb[:, ec, :],
                    start=(ec == 0), stop=False,
                )
            nc.tensor.matmul(acc, bs_sb, ones, start=False, stop=True)
            ts_sb = sb.tile([H, TPC], F32)
            nc.vector.tensor_copy(ts_sb, acc)
            nc.sync.dma_start(out=ts[:, :], in_=ts_sb)
    nc.finalize()
    return nc


def _build_phase_b(debug=False):
    """Per-core sparse attention for 512 queries x 6 heads (see header)."""
    _patch_tile_drain()
    nc = bacc.Bacc()
    if debug:
        davs = nc.declare_dram_parameter("davs", [DH + 1, 3, QB], F32, isOutput=True)
        drin = nc.declare_dram_parameter("drin", [1, 3, QB], F32, isOutput=True)
        drin0 = nc.declare_dram_parameter("drin0", [1, 3, QB], F32, isOutput=True)
        drbs = nc.declare_dram_parameter("drbs", [DH, 3, QB], F32, isOutput=True)
        dattnT = nc.declare_dram_parameter("dattnT", [DH, QB], F32, isOutput=True)
    # host-prepared, partition-major, bf16
    xTb = nc.declare_dram_parameter("xTb", [128, ECH, BT], BF16, isOutput=False)
    xTq = nc.declare_dram_parameter("xTq", [128, ECH, QB], BF16, isOutput=False)
    xTk = nc.declare_dram_parameter("xTk", [128, ECH, HPG * TOPK], BF16, isOutput=False)
    xTg = nc.declare_dram_parameter("xTg", [128, ECH, NG], BF16, isOutput=False)
    # per-type weight slices for this head group (384 = 6 heads x 64 dims)
    WqTg = nc.declare_dram_parameter("WqTg", [128, ECH, 384], BF16, isOutput=False)
    WkTg = nc.declare_dram_parameter("WkTg", [128, ECH, 384], BF16, isOutput=False)
    WvTg = nc.declare_dram_parameter("WvTg", [128, ECH, 384], BF16, isOutput=False)
    bqg = nc.declare_dram_parameter("bqg", [1, 384], BF16, isOutput=False)
    bkg = nc.declare_dram_parameter("bkg", [1, 384], BF16, isOutput=False)
    bvg = nc.declare_dram_parameter("bvg", [1, 384], BF16, isOutput=False)
    # stacked [Wk_h | Wv_h] (even local head) / [Wv_h | Wk_h] (odd) for the
    # compact top-k / global projections
    Wkv2 = nc.declare_dram_parameter("Wkv2", [128, ECH, HPG, 128], BF16, isOutput=False)
    bkv2 = nc.declare_dram_parameter("bkv2", [1, HPG, 128], BF16, isOutput=False)
    # Wo^T/3 slices: wo[dh, j, dc, :] over this group's heads
    Wo3 = nc.declare_dram_parameter("Wo3", [DH, HPG, ECH, 128], BF16, isOutput=False)
    M8 = nc.declare_dram_parameter("M8", [128, BCH, QB], BF16, isOutput=False)
    yT = nc.declare_dram_parameter("yT", [128, ECH, QB], F32, isOutput=True)

    with TileContext(nc) as tc, nc.allow_low_precision(reason="bf16 validated vs reference"):
        with tc.tile_pool(name="perm", bufs=1) as perm:
            kT_sb = perm.tile([128, HPG // 2, BT], BF16)       # 2 heads / 128 parts
            qT_sb = perm.tile([128, HPG // 2, QB], BF16)
            ktk_sb = perm.tile([128, HPG // 2, TOPK], BF16)
            vtkT_sb = perm.tile([128, HPG // 2, TOPK], BF16)   # staging (pre-transpose)
            vtk_sb = perm.tile([128, TOPK // 128, HPG, DH + 1], BF16)
            kg_sb = perm.tile([128, HPG // 2, NG], BF16)
            vgT_sb = perm.tile([128, HPG // 2, NG], BF16)      # staging
            vg_sb = perm.tile([NG, HPG, DH + 1], BF16)
            v_sb = perm.tile([128, BCH, HPG, DH + 1], BF16)
            M8_sb = perm.tile([128, BCH, QB], BF16)
            attnT = perm.tile([DH, HPG, QB], BF16)
            yT_sb = perm.tile([128, ECH, QB], F32)
            wo_sb = perm.tile([DH, HPG, ECH, 128], BF16)
            bqg_sb = perm.tile([1, 384], BF16)
            bkg_sb = perm.tile([1, 384], BF16)
            bvg_sb = perm.tile([1, 384], BF16)
            bkv2_sb = perm.tile([1, HPG, 128], BF16)
            ones = perm.tile([1, QB], BF16)
            id128 = perm.tile([128, 128], BF16)
            nc.vector.memset(ones, 1.0)
            make_identity(nc, id128)
            nc.vector.memset(v_sb[:, :, :, DH : DH + 1], 1.0)
            nc.vector.memset(vtk_sb[:, :, :, DH : DH + 1], 1.0)
            nc.vector.memset(vg_sb[:, :, DH : DH + 1], 1.0)
            nc.sync.dma_start(out=M8_sb, in_=M8[:, :, :])
            nc.sync.dma_start(out=bqg_sb, in_=bqg[:, :])
            nc.sync.dma_start(out=bkg_sb, in_=bkg[:, :])
            nc.sync.dma_start(out=bvg_sb, in_=bvg[:, :])
            nc.sync.dma_start(out=bkv2_sb, in_=bkv2[:, :, :])
            nc.sync.dma_start(out=wo_sb, in_=Wo3[:, :, :, :])

            with (
                tc.tile_pool(name="xin", bufs=1) as xin,
                tc.tile_pool(name="pj_ps", bufs=4, space="PSUM") as pj_ps,
                tc.tile_pool(name="pg_ps", bufs=2, space="PSUM") as pg_ps,
                tc.tile_pool(name="pt_ps", bufs=2, space="PSUM") as pt_ps,
            ):
                xTb_sb = xin.tile([128, ECH, BT], BF16)
                xTq_sb = xin.tile([128, ECH, QB], BF16)
                xTk_sb = xin.tile([128, ECH, HPG * TOPK], BF16)
                xTg_sb = xin.tile([128, ECH, NG], BF16)
                WqTg_sb = xin.tile([128, ECH, 384], BF16)
                WkTg_sb = xin.tile([128, ECH, 384], BF16)
                WvTg_sb = xin.tile([128, ECH, 384], BF16)
                Wkv2_sb = xin.tile([128, ECH, HPG, 128], BF16)
                # Q first (smallest working set) so the PE starts while the
                # larger K/V/band transfers stream in.
                for ec in range(ECH):
                    nc.sync.dma_start(out=WqTg_sb[:, ec, :], in_=WqTg[:, ec])
                    nc.sync.dma_start(out=xTq_sb[:, ec, :], in_=xTq[:, ec])
                for ec in range(ECH):
                    nc.sync.dma_start(out=WkTg_sb[:, ec, :], in_=WkTg[:, ec])
                    nc.sync.dma_start(out=xTb_sb[:, ec, 0:512], in_=xTb[:, ec, 0:512])
                    nc.sync.dma_start(out=xTb_sb[:, ec, 512:BT], in_=xTb[:, ec, 512:BT])
                for ec in range(ECH):
                    nc.sync.dma_start(out=WvTg_sb[:, ec, :], in_=WvTg[:, ec])
                    nc.sync.dma_start(out=xTk_sb[:, ec, :], in_=xTk[:, ec])
                    nc.sync.dma_start(out=Wkv2_sb[:, ec], in_=Wkv2[:, ec])
                nc.sync.dma_start(out=xTg_sb, in_=xTg[:, :, :])

                # ---- Q^T [dims, 512], then band K^T [dims, 1024] ----
                for dc in range(HPG // 2):
                    ds = slice(128 * dc, 128 * (dc + 1))
                    qp = pj_ps.tile([128, 512], F32, tag="pj")
                    for ec in range(ECH):
                        nc.tensor.matmul(
                            qp, WqTg_sb[:, ec, ds], xTq_sb[:, ec, :],
                            start=(ec == 0), stop=False,
                        )
                    nc.tensor.matmul(
                        qp, bqg_sb[:, ds], ones, start=False, stop=True
                    )
                    nc.any.tensor_copy(qT_sb[:, dc, :], qp)
                for dc in range(HPG // 2):
                    ds = slice(128 * dc, 128 * (dc + 1))
                    for half in range(2):
                        t0 = 512 * half
                        kp = pj_ps.tile([128, 512], F32, tag="pj")
                        for ec in range(ECH):
                            nc.tensor.matmul(
                                kp, WkTg_sb[:, ec, ds],
                                xTb_sb[:, ec, t0 : t0 + 512],
                                start=(ec == 0), stop=False,
                            )
                        nc.tensor.matmul(
                            kp, bkg_sb[:, ds], ones, start=False, stop=True
                        )
                        nc.any.tensor_copy(kT_sb[:, dc, t0 : t0 + 512], kp)

                # ---- band V [1024 tokens, 6 heads x 64] ----
                for tcn in range(BCH):
                    t0 = 128 * tcn
                    vp = pj_ps.tile([128, HPG * DH], F32, tag="pj")
                    for ec in range(ECH):
                        nc.tensor.matmul(
                            vp, xTb_sb[:, ec, t0 : t0 + 128],
                            WvTg_sb[:, ec, :],
                            start=(ec == 0), stop=False,
                        )
                    nc.tensor.matmul(
                        vp, ones[:, :128], bvg_sb,
                        start=False, stop=True,
                    )
                    nc.any.tensor_copy(
                        v_sb[:, tcn, :, 0:DH],
                        vp.rearrange("p (h d) -> p h d", d=DH),
                    )

                # ---- compact top-k + global K/V per head (stacked K|V) ----
                for j in range(HPG):
                    hp = (j % 2) * 64
                    dc = j // 2
                    sp = pj_ps.tile([128, TOPK], F32, tag="pj")
                    sg = pg_ps.tile([128, NG], F32, tag="pg")
                    for ec in range(ECH):
                        nc.tensor.matmul(
                            sp, Wkv2_sb[:, ec, j, :],
                            xTk_sb[:, ec, TOPK * j : TOPK * (j + 1)],
                            start=(ec == 0), stop=False,
                        )
                        nc.tensor.matmul(
                            sg, Wkv2_sb[:, ec, j, :], xTg_sb[:, ec, :],
                            start=(ec == 0), stop=False,
                        )
                    nc.tensor.matmul(
                        sp, bkv2_sb[:, j, :], ones[:, :TOPK], start=False, stop=True
                    )
                    nc.tensor.matmul(
                        sg, bkv2_sb[:, j, :], ones[:, :NG], start=False, stop=True
                    )
                    # even j: K at rows 0:64, V^T at 64:128; odd j: swapped
                    nc.any.tensor_copy(ktk_sb[hp : hp + 64, dc, :], sp[hp : hp + 64, :])
                    nc.any.tensor_copy(kg_sb[hp : hp + 64, dc, :], sg[hp : hp + 64, :])
                    vq = 64 - hp
                    nc.any.tensor_copy(
                        vtkT_sb[vq : vq + 64, dc, :], sp[vq : vq + 64, :]
                    )
                    nc.any.tensor_copy(
                        vgT_sb[vq : vq + 64, dc, :], sg[vq : vq + 64, :]
                    )

                # ---- transpose V^T staging into [token, dh] layout ----
                for j in range(HPG):
                    vq = 64 - (j % 2) * 64
                    dc = j // 2
                    idsl = id128[vq : vq + 64, vq : vq + 64]
                    for c in range(TOPK // 128):
                        tp = pt_ps.tile([128, 64], BF16, tag="pt")
                        nc.tensor.transpose(
                            tp, vtkT_sb[vq : vq + 64, dc, 128 * c : 128 * (c + 1)],
                            idsl,
                        )
                        nc.any.tensor_copy(vtk_sb[:, c, j, 0:DH], tp)
                    tg = pt_ps.tile([128, 64], BF16, tag="pt")
                    nc.tensor.transpose(tg[0:NG, :], vgT_sb[vq : vq + 64, dc, :NG], idsl)
                    nc.any.tensor_copy(vg_sb[:, j, 0:DH], tg[0:NG, :])

            # ---- per-head attention ----
            with (
                tc.tile_pool(name="attn", bufs=2) as attn,
                tc.tile_pool(name="nrm", bufs=2) as nrm,
                tc.tile_pool(name="st_ps", bufs=2, space="PSUM") as st_ps,
                tc.tile_pool(name="av_ps", bufs=1, space="PSUM") as av_ps,
            ):
                for j in range(HPG):
                    hp = (j % 2) * 64
                    dc = j // 2
                    kTh = kT_sb[hp : hp + 64, dc, :]
                    qTh = qT_sb[hp : hp + 64, dc, :]
                    ET = attn.tile([128, BCH, QB], BF16, tag="ET")
                    for rnd in range(BCH // 2):
                        stp = st_ps.tile([128, 2, QB], F32, tag="st")
                        for i in range(2):
                            cc = 2 * rnd + i
                            nc.tensor.matmul(
                                stp[:, i, :], kTh[:, 128 * cc : 128 * (cc + 1)],
                                qTh, start=True, stop=True,
                            )
                        nc.scalar.activation(
                            ET[:, 2 * rnd : 2 * rnd + 2, :], stp, Exp, scale=SCALE
                        )
                        nc.vector.tensor_mul(
                            ET[:, 2 * rnd : 2 * rnd + 2, :],
                            ET[:, 2 * rnd : 2 * rnd + 2, :],
                            M8_sb[:, 2 * rnd : 2 * rnd + 2, :],
                        )
                    # top-k scores
                    stk = st_ps.tile([128, 2, QB], F32, tag="st")
                    for c in range(2):
                        nc.tensor.matmul(
                            stk[:, c, :], ktk_sb[hp : hp + 64, dc, 128 * c : 128 * (c + 1)],
                            qTh, start=True, stop=True,
                        )
                    ETk = attn.tile([128, 2, QB], BF16, tag="ETk")
                    nc.scalar.activation(ETk, stk, Exp, scale=SCALE)
                    # global scores
                    stg = st_ps.tile([128, 2, QB], F32, tag="st")
                    nc.tensor.matmul(
                        stg[0:NG, 0, :], kg_sb[hp : hp + 64, dc, :], qTh,
                        start=True, stop=True,
                    )
                    ETg = attn.tile([NG, QB], BF16, tag="ETg")
                    nc.scalar.activation(ETg, stg[0:NG, 0, :], Exp, scale=SCALE)
                    # attention-weighted values; row 64 = branch denominators
                    av = av_ps.tile([DH + 1, 3, QB], F32, tag="av")
                    for tcn in range(BCH):
                        nc.tensor.matmul(
                            av[:, 0, :], v_sb[:, tcn, j, :], ET[:, tcn, :],
                            start=(tcn == 0), stop=(tcn == BCH - 1),
                        )
                    for c in range(TOPK // 128):
                        nc.tensor.matmul(
                            av[:, 1, :], vtk_sb[:, c, j, :], ETk[:, c, :],
                            start=(c == 0), stop=(c == TOPK // 128 - 1),
                        )
                    nc.tensor.matmul(
                        av[:, 2, :], vg_sb[:, j, :], ETg, start=True, stop=True
                    )
                    # normalize: av -> SBUF (frees PSUM), 1/denom on DVE,
                    # DMA the recip row to partition 0 (partition_broadcast
                    # only reads partition 0 on HW), broadcast on gpsimd,
                    # weighted sum on gpsimd. Entirely off the PE.
                    avs = nrm.tile([DH + 1, 3, QB], F32, tag="avs")
                    nc.scalar.copy(avs, av)
                    dn0 = nrm.tile([1, 3, QB], F32, tag="dn0")
                    nc.sync.dma_start(out=dn0, in_=avs[DH : DH + 1, :, :])
                    dbs = nrm.tile([DH, 3, QB], F32, tag="dbs")
                    for b in range(3):
                        nc.gpsimd.partition_broadcast(dbs[:, b, :], dn0[:, b, :])
                    rbs = nrm.tile([DH, 3, QB], F32, tag="rbs")
                    nc.vector.reciprocal_approx_fast(rbs, dbs)
                    ta = nrm.tile([DH, QB], F32, tag="ta")
                    tb = nrm.tile([DH, QB], F32, tag="tb")
                    nc.vector.tensor_mul(ta, avs[0:DH, 0, :], rbs[:, 0, :])
                    nc.vector.tensor_mul(tb, avs[0:DH, 1, :], rbs[:, 1, :])
                    nc.vector.tensor_add(ta, ta, tb)
                    nc.vector.tensor_mul(tb, avs[0:DH, 2, :], rbs[:, 2, :])
                    nc.vector.tensor_add(attnT[:, j, :], ta, tb)
                    if debug and j == 0:
                        nc.sync.dma_start(out=davs[:, :, :], in_=avs)
                        nc.sync.dma_start(out=drin[:, :, :], in_=dn0)
                        nc.sync.dma_start(out=drin0[:, :, :], in_=dn0)
                        nc.sync.dma_start(out=drbs[:, :, :], in_=rbs)
                        dat = nrm.tile([DH, QB], F32, tag="dat")
                        nc.vector.tensor_copy(dat, attnT[:, 0, :])
                        nc.sync.dma_start(out=dattnT[:, :], in_=dat)

            # ---- partial out-projection (Wo/3 folded on host) ----
            with tc.tile_pool(name="yt_ps", bufs=2, space="PSUM") as yt_ps:
                for ddc in range(ECH):
                    yp = yt_ps.tile([128, QB], F32, tag="yt")
                    for j in range(HPG):
                        nc.tensor.matmul(
                            yp, wo_sb[:, j, ddc, :], attnT[:, j, :],
                            start=(j == 0), stop=(j == HPG - 1),
                        )
                    nc.any.tensor_copy(yT_sb[:, ddc, :], yp)
                    nc.sync.dma_start(out=yT[:, ddc, :], in_=yT_sb[:, ddc, :])
    nc.finalize()
    return nc


_PROGS = {}
TRACE = False
LAST_EXEC_NS = {}


def _get_progs():
    if "a" not in _PROGS:
        _PROGS["a"] = _build_phase_a()
        _PROGS["b"] = _build_phase_b()
    return _PROGS["a"], _PROGS["b"]


def _pm(arr, dtype):
    """[768, T] -> partition-major [128, 6, T] contiguous."""
    d, t = arr.shape
    assert d == D
    return np.ascontiguousarray(
        arr.reshape(ECH, 128, t).transpose(1, 0, 2).astype(dtype)
    )


def _inputs_b(inputs, xT, topk_idx):
    """Build the 8 per-core phase-B input maps."""
    bf = ml_dtypes.bfloat16
    WqT = inputs["Wq"].T.astype(np.float32)
    WkT = inputs["Wk"].T.astype(np.float32)
    WvT = inputs["Wv"].T.astype(np.float32)
    WoT3 = (inputs["Wo"].T / 3.0).astype(np.float32)      # [768 in, 768 out]
    bq, bk, bv = (np.asarray(inputs[k], np.float32) for k in ("bq", "bk", "bv"))

    grp = []
    for g in range(HG):
        hs = slice(HPG * g * DH, HPG * (g + 1) * DH)
        wq, wk, wv = WqT[:, hs], WkT[:, hs], WvT[:, hs]   # [768, 384]
        Wkv2 = np.empty((D, HPG, 128), np.float32)
        bkv2 = np.empty((1, HPG, 128), np.float32)
        for j in range(HPG):
            js = slice(j * DH, (j + 1) * DH)
            if j % 2 == 0:
                Wkv2[:, j, 0:64], Wkv2[:, j, 64:128] = wk[:, js], wv[:, js]
                bkv2[0, j, 0:64], bkv2[0, j, 64:128] = bk[hs][js], bv[hs][js]
            else:
                Wkv2[:, j, 0:64], Wkv2[:, j, 64:128] = wv[:, js], wk[:, js]
                bkv2[0, j, 0:64], bkv2[0, j, 64:128] = bv[hs][js], bk[hs][js]
        wo = np.empty((DH, HPG, ECH, 128), np.float32)
        for j in range(HPG):
            for dcc in range(ECH):
                wo[:, j, dcc, :] = WoT3[
                    HPG * g * DH + j * DH : HPG * g * DH + (j + 1) * DH,
                    128 * dcc : 128 * (dcc + 1),
                ]
        xk = np.concatenate(
            [xT[:, topk_idx[HPG * g + j]] for j in range(HPG)], axis=1
        )
        grp.append(
            dict(
                WqTg=_pm(np.ascontiguousarray(wq), bf),
                WkTg=_pm(np.ascontiguousarray(wk), bf),
                WvTg=_pm(np.ascontiguousarray(wv), bf),
                bqg=np.ascontiguousarray(bq[hs][None, :]).astype(bf),
                bkg=np.ascontiguousarray(bk[hs][None, :]).astype(bf),
                bvg=np.ascontiguousarray(bv[hs][None, :]).astype(bf),
                Wkv2=_pm(Wkv2.reshape(D, HPG * 128), bf).reshape(128, ECH, HPG, 128),
                bkv2=bkv2.astype(bf),
                Wo3=np.ascontiguousarray(wo.astype(bf)),
                xTk=_pm(xk, bf),
            )
        )

    xTg_pm = _pm(xT[:, :NG], bf)
    p = np.arange(128)[:, None, None]
    kk = np.arange(BCH)[None, :, None]
    sl = np.arange(QB)[None, None, :]
    in_b = []
    for c in range(NCORES):
        qb, g = c // HG, c % HG
        s0 = min(max(4 * qb - 2, 0), S // 128 - BCH)
        M8 = (np.abs(128 * s0 + 128 * kk + p - (QB * qb + sl)) <= LWH).astype(bf)
        in_b.append(
            dict(
                grp[g],
                xTb=_pm(xT[:, 128 * s0 : 128 * s0 + BT], bf),
                xTq=_pm(xT[:, QB * qb : QB * (qb + 1)], bf),
                xTg=xTg_pm,
                M8=np.ascontiguousarray(M8),
            )
        )
    return in_b


def kernel(**inputs):
    x = np.asarray(inputs["x"][0], np.float32)            # [S, D]
    xT = np.ascontiguousarray(x.T)                        # [D, S]
    nc_a, nc_b = _get_progs()

    # ---- phase A: token scores, sharded over 8 cores ----
    WsT_pm = _pm(np.ascontiguousarray(inputs["Ws"].T, np.float32), np.float32)
    bs_row = np.ascontiguousarray(inputs["bs"][None, :], np.float32)
    TPC = S // NCORES
    in_a = [
        {
            "xTa": _pm(xT[:, TPC * c : TPC * (c + 1)], np.float32),
            "WsT": WsT_pm,
            "bs_row": bs_row,
        }
        for c in range(NCORES)
    ]
    ra = run_bass_kernel_spmd(nc_a, in_a, list(range(NCORES)), trace=TRACE)
    ts = np.concatenate([r["ts"] for r in ra.results], axis=1)  # [H, S]
    LAST_EXEC_NS["phase_a"] = ra.exec_time_ns

    # f32r rounds scores by up to ~2^-11 * |score|; re-evaluate tokens near
    # each head's top-k threshold exactly so the selection matches fp32.
    Ws32 = np.asarray(inputs["Ws"], np.float32)
    bs32 = np.asarray(inputs["bs"], np.float32)
    topk_idx = []
    for h in range(H):
        order = np.argpartition(-ts[h], TOPK)
        thresh = ts[h][order[TOPK - 1]]
        margin = 0.02
        cand = np.nonzero(np.abs(ts[h] - thresh) <= margin)[0]
        tsf = ts[h].copy()
        tsf[cand] = Ws32[h] @ xT[:, cand] + bs32[h]
        topk_idx.append(np.argpartition(-tsf, TOPK)[:TOPK])
    in_b = _inputs_b(inputs, xT, topk_idx)
    res = run_bass_kernel_spmd(nc_b, in_b, list(range(NCORES)), trace=TRACE)
    LAST_EXEC_NS["phase_b"] = res.exec_time_ns

    bo = np.asarray(inputs["bo"], np.float32)
    out = np.empty((S, D), np.float32)
    for qb in range(NQB):
        ypm = res.results[2 * qb]["yT"] + res.results[2 * qb + 1]["yT"]
        yfull = ypm.transpose(1, 0, 2).reshape(D, QB)     # [768, 512]
        out[QB * qb : QB * (qb + 1)] = yfull.T + bo
    return out.reshape(1, S, D)
